# revision 1
# baseline (speedup 1.0000x reference)
"""Trainium2 Bass kernel for the DND retrieval problem.

Full (unsharded) inputs in, full output out. Internally: data-parallel over
the batch dim B=64 across 8 NeuronCores (8 batch elements per core), one
SPMD Bass program.

Per-core program (Bc=8, H=8, K=V=256, L=2048):
  q = (query @ Wq + bq)                      -> qT [k, (b,h)] via PE
  scoresT[(b,h), l] = sum_k qT[k,bh] keysT[k,l]   (keys PE-transposed on chip,
                                                   rpe folded in on copyback)
  softmax over l (free dim): additive -1e30 mask from runtime `steps`,
  reduce_max(negate) -> ACT Exp(bias=-max, accum_out=rowsum) -> recip
  read[(b,h), v] = sum_l w[l,bh] vals[l,v]   (w PE-transposed back; PSUM
                                              accumulation across all of L)
  out[b, :] = readT.T @ Wa + ba

Softmax rows use a dense layout: partition 8*b + h (rows 0..63). See _emit's
docstring for the scores / read matmul structure (f32r matmuls are restricted
to PSUM partition 0, which shapes both).
"""

import numpy as np

import concourse.bacc as bacc
import concourse.bass as bass
import concourse.mybir as mybir
import concourse.tile as tile
from concourse.bass_utils import run_bass_kernel_spmd
from concourse.masks import make_identity

F32 = mybir.dt.float32
F32R = mybir.dt.float32r  # PE fast-fp32 path: 1 cycle/row at N>=256 (vs 4 for fp32)
I32 = mybir.dt.int32

import os
ABLATE = os.environ.get("BASS_ABLATE", "")  # timing experiments only


def _r(ap):
    return ap.bitcast(F32R)

L = 2048
B = 64
K = 256
V = 256
H = 8
NCORES = 8
BC = B // NCORES          # 8 batch elements per core
NIT = 8                   # streaming iterations
SL = L // NIT             # 256 l-rows per iteration (2 x 128 subchunks)
NEG = -1.0e30


def _ap(tensor_ap, offset_elems, dims):
    """Build a raw AP on the same tensor with explicit [step, count] dims."""
    return bass.AP(tensor=tensor_ap.tensor, offset=offset_elems, ap=dims)


def _bcast_free(ap2d, n):
    """Append a broadcast (step 0) innermost free dim of size n."""
    return bass.AP(tensor=ap2d.tensor, offset=ap2d.offset, ap=[*ap2d.ap, [0, n]])


def build_nc():
    nc = bacc.Bacc("TRN2", target_bir_lowering=False)

    t_query = nc.dram_tensor("query", [BC, K], F32, kind="ExternalInput").ap()
    t_keys = nc.dram_tensor("keys", [L, BC, K], F32, kind="ExternalInput").ap()
    t_vals = nc.dram_tensor("vals", [L, BC, V], F32, kind="ExternalInput").ap()
    t_rpe = nc.dram_tensor("rpe", [L, BC], F32, kind="ExternalInput").ap()
    t_wq = nc.dram_tensor("wq", [K, H * K], F32, kind="ExternalInput").ap()
    t_bq = nc.dram_tensor("bq", [H * K], F32, kind="ExternalInput").ap()
    t_wa = nc.dram_tensor("wa", [H * V, V], F32, kind="ExternalInput").ap()
    t_ba = nc.dram_tensor("ba", [V], F32, kind="ExternalInput").ap()
    t_steps = nc.dram_tensor("steps", [BC], I32, kind="ExternalInput").ap()
    t_out = nc.dram_tensor("out", [BC, V], F32, kind="ExternalOutput").ap()

    with tile.TileContext(nc) as tc:
        _emit(nc, tc, t_query, t_keys, t_vals, t_rpe, t_wq, t_bq, t_wa, t_ba,
              t_steps, t_out)
    nc.compile()
    return nc


def _emit(nc, tc, t_query, t_keys, t_vals, t_rpe, t_wq, t_bq, t_wa, t_ba,
          t_steps, t_out):
    """Dense layout v2: softmax row = 8*b + h (rows 0..63).

    scores:  one PSUM tile [64, 256] per l-iter; 16 serial f32r matmuls with a
             block-diagonal qT (64 cols, zeros outside batch b's 8 columns) —
             f32r matmuls may only target PSUM partition 0, so instead of col
             packing, all 64 (b,h) rows come from one matmul's column space.
    read:    computed TRANSPOSED: readT[v, (b,h)] accumulates in two PSUM
             tiles [128, 64] (v halves) with vals as the stationary operand
             and the 8-col w slices as moving; all 8 batches share each bank
             via disjoint free-dim column ranges in one accumulation group.
    """
    from contextlib import ExitStack
    ctx = ExitStack()
    with ctx:
        consts = ctx.enter_context(tc.tile_pool(name="consts", bufs=1))
        keysP = ctx.enter_context(tc.tile_pool(name="keysP", bufs=2))
        keysTP = ctx.enter_context(tc.tile_pool(name="keysTP", bufs=2))
        valsP = ctx.enter_context(tc.tile_pool(name="valsP", bufs=10))
        smallP = ctx.enter_context(tc.tile_pool(name="smallP", bufs=4))
        ptP = ctx.enter_context(tc.tile_pool(name="ptP", bufs=3, space="PSUM"))
        psP = ctx.enter_context(tc.tile_pool(name="psP", bufs=2, space="PSUM"))
        prP = ctx.enter_context(tc.tile_pool(name="prP", bufs=2, space="PSUM"))

        # ---------------- prologue ----------------
        ident = consts.tile([128, 128], F32, tag="ident")
        make_identity(nc, ident)
        ident_r = consts.tile([128, 128], F32R, tag="ident_r")
        nc.vector.tensor_copy(ident_r, ident)

        # first two keys tiles ahead of the weight loads, one per DMA queue,
        # so both queues start on the critical 16.8MB keys stream immediately
        keys0 = keysP.tile([128, 2, BC, K], F32R, tag="keys")
        nc.sync.dma_start(
            out=keys0,
            in_=_r(t_keys[0:SL].rearrange("(s p) b k -> p s b k", s=2)))
        keys1 = keysP.tile([128, 2, BC, K], F32R, tag="keys")
        nc.gpsimd.dma_start(
            out=keys1,
            in_=_r(t_keys[SL:2 * SL].rearrange("(s p) b k -> p s b k", s=2)))

        iota = consts.tile([64, L], F32, tag="iota")
        nc.gpsimd.iota(iota, pattern=[[1, L]], base=0, channel_multiplier=0,
                       allow_small_or_imprecise_dtypes=True)

        # weight/side loads via SWDGE (Pool queue)
        query_sb = consts.tile([BC, K], F32, tag="query")
        nc.gpsimd.dma_start(out=query_sb, in_=t_query)

        wmat = consts.tile([128, 2, H * K], F32, tag="wmat")
        nc.gpsimd.dma_start(out=wmat, in_=t_wq.rearrange("(a p) j -> p a j", a=2))

        bq_nat = consts.tile([16, 128], F32, tag="bq_nat")
        nc.gpsimd.dma_start(out=bq_nat, in_=t_bq.rearrange("(r q) -> r q", r=16))

        rpe_sb = consts.tile([128, 16, BC], F32, tag="rpe")
        nc.gpsimd.dma_start(out=rpe_sb, in_=t_rpe.rearrange("(t p) b -> p t b", t=16))

        ba_rep = consts.tile([128, V], F32, tag="ba_rep")
        nc.gpsimd.dma_start(out=ba_rep, in_=_ap(t_ba, 0, [[0, 128], [1, V]]))

        # hoisted: iteration 0's keys transposes, emitted BEFORE the qT
        # prologue so the PE stream starts as soon as keys0 lands instead of
        # stalling behind prologue matmuls that wait on the weight loads
        def emit_kt(keys_tile):
            kT = keysTP.tile([128, BC, 2, 2, 128], F32R, tag="keysT",
                             name="kT")
            for b in range(BC):
                for s in range(2):
                    pk = ptP.tile([128, 256], F32, tag="pt", name="pk")
                    for kc in range(2):
                        nc.tensor.transpose(
                            _r(pk[:, kc * 128:(kc + 1) * 128]),
                            keys_tile[:, s, b, kc * 128:(kc + 1) * 128],
                            ident_r)
                    cb = nc.vector.tensor_copy if b % 2 == 0 else nc.scalar.copy
                    cb(kT[:, b, :, s, :], pk.rearrange("p (a c) -> p a c", a=2))
            return kT

        kT0 = emit_kt(keys0)

        # steps replicated to the dense layout: partition 8b+h <- steps[b]
        sti = consts.tile([64, 1], I32, tag="sti")
        for b in range(BC):
            nc.gpsimd.dma_start(
                out=sti[8 * b:8 * b + 8, :],
                in_=_ap(t_steps, b, [[0, 8], [0, 1]]))
        steps_sb = consts.tile([64, 1], F32, tag="steps")
        nc.vector.tensor_copy(steps_sb, sti)

        # queryT [k, b] via PE transpose of query [b, k]
        queryT = consts.tile([128, 2, BC], F32, tag="queryT")
        for half in range(2):
            pq = ptP.tile([128, 256], F32, tag="pt")
            nc.tensor.transpose(
                pq[:, :BC], query_sb[:, half * 128:(half + 1) * 128],
                ident[:BC, :BC])
            nc.any.tensor_copy(queryT[:, half, :], pq[:, :BC])

        # bqT [kout, (h,kc)] via PE transpose
        bq_sb = consts.tile([128, 16], F32, tag="bq_sb")
        pb = ptP.tile([128, 256], F32, tag="pt")
        nc.tensor.transpose(pb[:, :16], bq_nat, ident[:16, :16])
        nc.any.tensor_copy(bq_sb, pb[:, :16])

        # block-diagonal qT: [kout(128), kc, b, 64 cols]; col 8b+h holds
        # q[b,h,kout], all other columns zero (so one matmul per (b,kc)
        # accumulates into all 64 (b,h) score rows without cross-terms)
        qTblk = consts.tile([128, 2, BC, 64], F32R, tag="qTblk")
        nc.vector.memset(qTblk.bitcast(F32), 0.0)
        for kc in range(2):
            for h in range(H):
                pq2 = ptP.tile([128, 256], F32, tag="pt")
                for kin in range(2):
                    col0 = h * K + kc * 128
                    nc.tensor.matmul(
                        pq2[:, :BC],
                        lhsT=wmat[:, kin, col0:col0 + 128],
                        rhs=queryT[:, kin, :],
                        start=(kin == 0), stop=(kin == 1),
                    )
                # scatter b -> column 8b+h of batch-b's block (stride 72)
                nc.scalar.activation(
                    _ap(qTblk, kc * 512 + h, [[qTblk.ap[0][0], 128], [72, BC]]),
                    pq2[:, :BC],
                    mybir.ActivationFunctionType.Identity,
                    bias=bq_sb[:, h * 2 + kc:h * 2 + kc + 1], scale=1.0)

        # additive -1e30 mask from runtime steps
        addmask = consts.tile([64, L], F32, tag="addmask")
        nc.vector.tensor_scalar(
            out=addmask, in0=iota, scalar1=steps_sb, scalar2=NEG,
            op0=mybir.AluOpType.is_ge, op1=mybir.AluOpType.mult)

        scoresT = consts.tile([64, L], F32, tag="scoresT")
        runmax = consts.tile([64, 2 * NIT], F32, tag="runmax")

        # ---------------- phase 1: stream keys, build scoresT ----------------
        for it in range(NIT):
            if it == 0:
                keys_tile = keys0
            elif it == 1:
                keys_tile = keys1
            else:
                keys_tile = keysP.tile([128, 2, BC, K], F32R, tag="keys")
                eng = nc.sync if it % 2 == 0 else nc.gpsimd
                eng.dma_start(
                    out=keys_tile,
                    in_=_r(t_keys[it * SL:(it + 1) * SL].rearrange(
                        "(s p) b k -> p s b k", s=2)))

            kT = kT0 if it == 0 else emit_kt(keys_tile)

            # rpeT chunk [8b+h, l] = rpe[l, b], one per 128-l subchunk
            prT = {}
            for s in range(2):
                rr = smallP.tile([128, 64], F32, tag="rr")
                src = rpe_sb[:, it * 2 + s, :]
                nc.vector.tensor_copy(
                    rr.rearrange("p (b j) -> p b j", b=BC),
                    _bcast_free(src, 8))
                pr_ = ptP.tile([128, 256], F32, tag="pt")
                nc.tensor.transpose(pr_[:64, :128], rr, ident)
                rT = smallP.tile([64, 128], F32, tag="rr")
                nc.any.tensor_copy(rT, pr_[:64, :128])
                prT[s] = rT

            # scores: 16 serial f32r matmuls into one [64, 256] PSUM tile
            pscore = psP.tile([64, SL], F32, tag="ps")
            n_mm = 2 * BC
            i_mm = 0
            for kc in range(2):
                for b in range(BC):
                    nc.tensor.matmul(
                        pscore,
                        lhsT=qTblk[:, kc, b, :],
                        rhs=kT[:, b, kc, :, :].rearrange("p s l -> p (s l)"),
                        start=(i_mm == 0), stop=(i_mm == n_mm - 1))
                    i_mm += 1

            for s in range(2):
                lo = it * SL + s * 128
                chunk = scoresT[:, lo:lo + 128]
                nc.vector.tensor_mul(
                    chunk, pscore[:, s * 128:(s + 1) * 128], prT[s])
                nc.vector.tensor_add(chunk, chunk, addmask[:, lo:lo + 128])
                nc.vector.reduce_max(
                    runmax[:, it * 2 + s:it * 2 + s + 1], chunk,
                    axis=mybir.AxisListType.X)

        # ---------------- softmax over l (free dim) ----------------
        # chunked exp (in place): wT transposes can start after chunk 0
        psums = consts.tile([64, 2 * NIT], F32, tag="psums")
        negmax = consts.tile([64, 1], F32, tag="negmax")
        nc.vector.reduce_max(negmax, runmax, axis=mybir.AxisListType.X,
                             negate=True)
        for ch in range(2 * NIT):
            lo = ch * 128
            nc.scalar.activation(scoresT[:, lo:lo + 128],
                                 scoresT[:, lo:lo + 128],
                                 mybir.ActivationFunctionType.Exp,
                                 bias=negmax, scale=1.0,
                                 accum_out=psums[:, ch:ch + 1])

        # Wa load (reuses the Wq slot), f32r for the projection matmuls
        wa_sb = consts.tile([128, 16, V], F32R, tag="wmat")
        nc.gpsimd.dma_start(
            out=wa_sb, in_=_r(t_wa.rearrange("(a p) j -> p a j", a=16)))

        # ---------------- phase 2: stream vals, accumulate readT --------------
        # readT[v, 8b+h] in two PSUM tiles (v halves); vals is the stationary
        # operand, w chunks the moving one; all 8 b's share each bank via
        # disjoint 8-column ranges inside one accumulation group
        preadT = [prP.tile([128, 64], F32, tag="pr", name=f"preadT{vh}")
                  for vh in range(2)]
        for it in range(2 * NIT):
            vals_tile = valsP.tile([128, BC, V], F32R, tag="vals")
            # alternate the two DMA queues (SP/HWDGE and Pool/SWDGE) so the
            # vals stream isn't serialized behind a single queue
            eng = nc.sync if it % 2 == 0 else nc.gpsimd
            eng.dma_start(
                out=vals_tile, in_=_r(t_vals[it * 128:(it + 1) * 128]))

            pw = ptP.tile([128, 256], F32, tag="pt")
            off = it * 128
            nc.tensor.transpose(pw[:, :64], scoresT[:, off:off + 128],
                                ident[:64, :64])
            w_sb = smallP.tile([128, 64], F32R, tag="wsb")
            nc.vector.tensor_copy(w_sb, pw[:, :64])
            for vh in range(2):
                for b in range(BC):
                    nc.tensor.matmul(
                        preadT[vh][:, 8 * b:8 * b + 8],
                        lhsT=vals_tile[:, b, vh * 128:(vh + 1) * 128],
                        rhs=w_sb[:, 8 * b:8 * b + 8],
                        start=(it == 0 and b == 0),
                        stop=(it == 2 * NIT - 1 and b == BC - 1),
                        skip_group_check=True)

        # softmax denominator (deferred: depends on every ACT exp chunk)
        sumexp = consts.tile([64, 1], F32, tag="sumexp")
        nc.vector.reduce_sum(sumexp, psums, axis=mybir.AxisListType.X)
        recip = consts.tile([64, 1], F32, tag="recip")
        nc.vector.reciprocal(recip, sumexp)

        # broadcast recip over the v partitions via a DRAM round-trip
        t_rtmp = nc.dram_tensor("rtmp", [64], F32, kind="Internal").ap()
        nc.gpsimd.dma_start(out=t_rtmp, in_=recip[:, 0:1])
        recip_rep = consts.tile([128, 64], F32, tag="recip_rep")
        nc.gpsimd.dma_start(out=recip_rep, in_=_ap(t_rtmp, 0, [[0, 128], [1, 64]]))

        # ---------------- epilogue: normalize + head-aggregate + store -------
        readT_sb = consts.tile([128, 2, 64], F32R, tag="readT_sb")
        for vh in range(2):
            nc.vector.tensor_mul(readT_sb[:, vh, :], preadT[vh], recip_rep)

        po = prP.tile([64, V], F32, tag="pr")
        n_mm = 2 * H
        i_mm = 0
        for h in range(H):
            for half in range(2):
                lhsT = _ap(readT_sb, half * 64 + h,
                           [[readT_sb.ap[0][0], 128], [8, BC]])
                nc.tensor.matmul(
                    po[:BC, :], lhsT=lhsT, rhs=wa_sb[:, h * 2 + half, :],
                    start=(i_mm == 0), stop=(i_mm == n_mm - 1))
                i_mm += 1
        out_sb = consts.tile([BC, V], F32, tag="out_sb")
        nc.vector.tensor_add(out_sb, po[:BC, :], ba_rep[:BC, :])
        nc.sync.dma_start(out=t_out, in_=out_sb)


_NC_CACHE = None


def _get_nc():
    global _NC_CACHE
    if _NC_CACHE is None:
        _NC_CACHE = build_nc()
    return _NC_CACHE


def make_in_maps(query, keys, vals, rpe_mod, Wq, bq, Wa, ba, steps):
    in_maps = []
    for c in range(NCORES):
        bs = slice(c * BC, (c + 1) * BC)
        in_maps.append({
            "query": np.ascontiguousarray(query[bs], dtype=np.float32),
            "keys": np.ascontiguousarray(keys[:, bs, :], dtype=np.float32),
            "vals": np.ascontiguousarray(vals[:, bs, :], dtype=np.float32),
            "rpe": np.ascontiguousarray(
                np.asarray(rpe_mod)[:, bs, 0], dtype=np.float32),
            "wq": np.ascontiguousarray(Wq, dtype=np.float32),
            "bq": np.ascontiguousarray(bq, dtype=np.float32),
            "wa": np.ascontiguousarray(Wa, dtype=np.float32),
            "ba": np.ascontiguousarray(ba, dtype=np.float32),
            "steps": np.ascontiguousarray(steps[bs], dtype=np.int32),
        })
    return in_maps


def kernel(query, keys, vals, rpe_mod, Wq, bq, Wa, ba, steps):
    query = np.asarray(query)
    keys = np.asarray(keys)
    vals = np.asarray(vals)
    rpe_mod = np.asarray(rpe_mod)
    Wq = np.asarray(Wq)
    bq = np.asarray(bq)
    Wa = np.asarray(Wa)
    ba = np.asarray(ba)
    steps = np.asarray(steps)

    nc = _get_nc()
    in_maps = make_in_maps(query, keys, vals, rpe_mod, Wq, bq, Wa, ba, steps)
    res = run_bass_kernel_spmd(nc, in_maps, core_ids=list(range(NCORES)))
    out = np.concatenate([r["out"] for r in res.results], axis=0)
    return out.astype(np.float32)



# revision 7
# speedup vs baseline: 2.0181x; 2.0181x over previous
"""Trainium2 Bass kernel for the DND retrieval problem.

Full (unsharded) inputs in, full output out. Data-parallel over batch B=64
across 8 NeuronCores (8 batch elements per core), one SPMD Bass program.

v2 design: every large operand ships as fp16 (half the HBM bytes of the f32
baseline; fp16's 10-bit mantissa keeps absmax rel err ~3e-3, well under the
2e-2 gate). keys are pre-transposed on the host to [k, l] layout so the
on-chip PE-transpose + copyback pipeline disappears. All tensors are
SBUF-resident, so every DMA is issued eagerly with no waits and the DMA
engines stream back-to-back.

Per-core program (Bc=8, H=8, K=V=256, L=2048):
  qTblk[k, 64]   block-diagonal q (col 8b+h) from wq/query (PE prologue)
  scoresT[bh, l] 16 fp16 matmuls per 256-l chunk into one [64,2048] PSUM tile
  softmax over l: scoresT = pscore*rpeT + addmask; global max; ACT Exp with
                  accum rowsums; recip folded back into scoresT (no DRAM
                  round-trip)
  readT[v, bh]   per 128-l chunk: PE transpose of w + 16 tiny (N=8) matmuls
                  accumulating in PSUM across all chunks
  out[b, :]      readT.T @ Wa + ba
"""

import numpy as np

import concourse.bacc as bacc
import concourse.bass as bass
import concourse.mybir as mybir
import concourse.tile as tile
from concourse.bass_utils import run_bass_kernel_spmd
from concourse.masks import make_identity

F32 = mybir.dt.float32
F16 = mybir.dt.float16

L = 2048
B = 64
K = 256
V = 256
H = 8
NCORES = 8
BC = B // NCORES          # 8 batch elements per core
NKC = 8                   # keys chunks (256 l each)
KCL = L // NKC            # 256
NVC = 16                  # vals chunks (128 l each)
NEG = -1.0e30


def _ap(tensor_ap, offset_elems, dims):
    """Build a raw AP on the same tensor with explicit [step, count] dims."""
    return bass.AP(tensor=tensor_ap.tensor, offset=offset_elems, ap=dims)


def build_nc():
    nc = bacc.Bacc("TRN2", target_bir_lowering=False)

    t_query = nc.dram_tensor("query", [BC, K], F32, kind="ExternalInput").ap()
    t_keysT = nc.dram_tensor("keysT", [NKC, 128, 2, BC, KCL], F16,
                             kind="ExternalInput").ap()
    t_vals = nc.dram_tensor("vals", [NVC, 128, BC, V], F16,
                            kind="ExternalInput").ap()
    t_rpeT = nc.dram_tensor("rpeT", [B, L], F16, kind="ExternalInput").ap()
    t_wq = nc.dram_tensor("wq", [K, H * K], F16, kind="ExternalInput").ap()
    t_bq = nc.dram_tensor("bq", [H * K], F32, kind="ExternalInput").ap()
    t_wa = nc.dram_tensor("wa", [H * V, V], F16, kind="ExternalInput").ap()
    t_ba = nc.dram_tensor("ba", [V], F32, kind="ExternalInput").ap()
    t_steps = nc.dram_tensor("stepsf", [B], F32, kind="ExternalInput").ap()
    t_out = nc.dram_tensor("out", [BC, V], F32, kind="ExternalOutput").ap()

    with tile.TileContext(nc) as tc:
        _emit(nc, tc, t_query, t_keysT, t_vals, t_rpeT, t_wq, t_bq, t_wa,
              t_ba, t_steps, t_out)
    nc.compile()
    return nc


def _emit(nc, tc, t_query, t_keysT, t_vals, t_rpeT, t_wq, t_bq, t_wa, t_ba,
          t_steps, t_out):
    from contextlib import ExitStack
    ctx = ExitStack()
    with ctx:
        consts = ctx.enter_context(tc.tile_pool(name="consts", bufs=1))
        keysP = ctx.enter_context(tc.tile_pool(name="keysP", bufs=NKC))
        valsP = ctx.enter_context(tc.tile_pool(name="valsP", bufs=NVC))
        wsbP = ctx.enter_context(tc.tile_pool(name="wsbP", bufs=NVC))
        psP = ctx.enter_context(tc.tile_pool(name="psP", bufs=1, space="PSUM"))
        ptP = ctx.enter_context(tc.tile_pool(name="ptP", bufs=1, space="PSUM"))
        prP = ctx.enter_context(tc.tile_pool(name="prP", bufs=2, space="PSUM"))
        poP = ctx.enter_context(tc.tile_pool(name="poP", bufs=1, space="PSUM"))

        # ------------- DMA issue: weights first, then keys, then vals -------
        # Big stream on the SP/HWDGE queue, in order; side loads on the
        # Pool/SWDGE queue. All tiles are persistent, so no DMA ever waits.
        wmat = consts.tile([128, 2, H * K], F16, tag="wmat")
        nc.sync.dma_start(out=wmat, in_=t_wq.rearrange("(a p) j -> p a j", a=2))

        keys_tiles = []
        for ch in range(NKC):
            kt = keysP.tile([128, 2, BC, KCL], F16, tag="keys")
            nc.sync.dma_start(out=kt, in_=t_keysT[ch])
            keys_tiles.append(kt)

        vals_tiles = []
        for vc in range(NVC):
            vt = valsP.tile([128, BC, V], F16, tag="vals")
            nc.sync.dma_start(out=vt, in_=t_vals[vc])
            vals_tiles.append(vt)

        query_sb = consts.tile([BC, K], F32, tag="query")
        nc.gpsimd.dma_start(out=query_sb, in_=t_query)
        bq_nat = consts.tile([16, 128], F32, tag="bq_nat")
        nc.gpsimd.dma_start(out=bq_nat, in_=t_bq.rearrange("(r q) -> r q", r=16))
        stepsf = consts.tile([64, 1], F32, tag="stepsf")
        nc.gpsimd.dma_start(out=stepsf, in_=_ap(t_steps, 0, [[1, 64], [0, 1]]))
        rpeT = consts.tile([64, L], F16, tag="rpeT")
        nc.gpsimd.dma_start(out=rpeT, in_=t_rpeT)
        ba_rep = consts.tile([BC, V], F32, tag="ba_rep")
        nc.gpsimd.dma_start(out=ba_rep, in_=_ap(t_ba, 0, [[0, BC], [1, V]]))
        wa_sb = consts.tile([128, 16, V], F16, tag="wa_sb")
        nc.gpsimd.dma_start(
            out=wa_sb, in_=t_wa.rearrange("(a p) j -> p a j", a=16))

        # ------------- prologue compute -------------
        ident = consts.tile([128, 128], F32, tag="ident")
        make_identity(nc, ident)

        iota = consts.tile([64, L], F32, tag="iota")
        nc.gpsimd.iota(iota, pattern=[[1, L]], base=0, channel_multiplier=0,
                       allow_small_or_imprecise_dtypes=True)

        # queryT [k, b] (fp16) via PE transpose of query [b, k]
        queryT = consts.tile([128, 2, BC], F16, tag="queryT")
        for half in range(2):
            pq = ptP.tile([128, 256], F32, tag="pt")
            nc.tensor.transpose(
                pq[:, :BC], query_sb[:, half * 128:(half + 1) * 128],
                ident[:BC, :BC])
            nc.any.tensor_copy(queryT[:, half, :], pq[:, :BC])

        # bqT [kout, (h,kc)] via PE transpose
        bq_sb = consts.tile([128, 16], F32, tag="bq_sb")
        pb = ptP.tile([128, 256], F32, tag="pt")
        nc.tensor.transpose(pb[:, :16], bq_nat, ident[:16, :16])
        nc.any.tensor_copy(bq_sb, pb[:, :16])

        # block-diagonal qT: [kout(128), kc, b, 64 cols]; col 8b+h holds
        # q[b,h,kout], other columns zero, so one matmul per (kc, b)
        # accumulates all 64 (b,h) score rows without cross-terms
        qTblk = consts.tile([128, 2, BC, 64], F16, tag="qTblk")
        nc.vector.memset(qTblk, 0.0)
        for kc in range(2):
            for h in range(H):
                pq2 = ptP.tile([128, 256], F32, tag="pt")
                for kin in range(2):
                    col0 = h * K + kc * 128
                    nc.tensor.matmul(
                        pq2[:, :BC],
                        lhsT=wmat[:, kin, col0:col0 + 128],
                        rhs=queryT[:, kin, :],
                        start=(kin == 0), stop=(kin == 1),
                    )
                # scatter b -> column 8b+h of batch-b's block (stride 72)
                nc.scalar.activation(
                    _ap(qTblk, kc * 512 + h, [[qTblk.ap[0][0], 128], [72, BC]]),
                    pq2[:, :BC],
                    mybir.ActivationFunctionType.Identity,
                    bias=bq_sb[:, h * 2 + kc:h * 2 + kc + 1], scale=1.0)

        # additive -1e30 mask from runtime steps
        addmask = consts.tile([64, L], F32, tag="addmask")
        nc.vector.tensor_scalar(
            out=addmask, in0=iota, scalar1=stepsf, scalar2=NEG,
            op0=mybir.AluOpType.is_ge, op1=mybir.AluOpType.mult)

        # ------------- scores: one [64, 2048] PSUM tile -------------
        pscore = psP.tile([64, L], F32, tag="ps")
        for ch in range(NKC):
            kt = keys_tiles[ch]
            n_mm = 2 * BC
            i_mm = 0
            sl = pscore[:, ch * KCL:(ch + 1) * KCL]
            for kc in range(2):
                for b in range(BC):
                    nc.tensor.matmul(
                        sl,
                        lhsT=qTblk[:, kc, b, :],
                        rhs=kt[:, kc, b, :],
                        start=(i_mm == 0), stop=(i_mm == n_mm - 1),
                        skip_group_check=True)
                    i_mm += 1

        # ------------- softmax over l (free dim) -------------
        scoresT = consts.tile([64, L], F32, tag="scoresT")
        nc.vector.tensor_mul(scoresT, pscore, rpeT)
        nc.vector.tensor_add(scoresT, scoresT, addmask)
        negmax = consts.tile([64, 1], F32, tag="negmax")
        nc.vector.reduce_max(negmax, scoresT, axis=mybir.AxisListType.X,
                             negate=True)
        psums = consts.tile([64, 4], F32, tag="psums")
        for c in range(4):
            lo = c * 512
            nc.scalar.activation(scoresT[:, lo:lo + 512],
                                 scoresT[:, lo:lo + 512],
                                 mybir.ActivationFunctionType.Exp,
                                 bias=negmax, scale=1.0,
                                 accum_out=psums[:, c:c + 1])
        sumexp = consts.tile([64, 1], F32, tag="sumexp")
        nc.vector.reduce_sum(sumexp, psums, axis=mybir.AxisListType.X)
        recip = consts.tile([64, 1], F32, tag="recip")
        nc.vector.reciprocal(recip, sumexp)
        # fold the softmax denominator into the weights in place
        nc.vector.tensor_scalar(
            out=scoresT, in0=scoresT, scalar1=recip, scalar2=None,
            op0=mybir.AluOpType.mult)

        # ------------- read: accumulate readT[v, bh] over all l -------------
        preadT = [prP.tile([128, 64], F32, tag="pr", name=f"preadT{vh}")
                  for vh in range(2)]
        for vc in range(NVC):
            vt = vals_tiles[vc]
            pwt = ptP.tile([128, 256], F32, tag="pt", name="pwt")
            pw = pwt[:, :64]
            off = vc * 128
            nc.tensor.transpose(pw, scoresT[:, off:off + 128],
                                ident[:64, :64])
            w_sb = wsbP.tile([128, 64], F16, tag="wsb")
            cb = nc.vector.tensor_copy if vc % 2 == 0 else nc.scalar.copy
            cb(w_sb, pw)
            for vh in range(2):
                for b in range(BC):
                    nc.tensor.matmul(
                        preadT[vh][:, 8 * b:8 * b + 8],
                        lhsT=vt[:, b, vh * 128:(vh + 1) * 128],
                        rhs=w_sb[:, 8 * b:8 * b + 8],
                        start=(vc == 0 and b == 0),
                        stop=(vc == NVC - 1 and b == BC - 1),
                        skip_group_check=True)

        # ------------- epilogue: head aggregation + store -------------
        readT_sb = consts.tile([128, 2, 64], F16, tag="readT_sb")
        for vh in range(2):
            nc.vector.tensor_copy(readT_sb[:, vh, :], preadT[vh])

        po = poP.tile([64, V], F32, tag="po")
        n_mm = 2 * H
        i_mm = 0
        for h in range(H):
            for half in range(2):
                lhsT = _ap(readT_sb, half * 64 + h,
                           [[readT_sb.ap[0][0], 128], [8, BC]])
                nc.tensor.matmul(
                    po[:BC, :], lhsT=lhsT, rhs=wa_sb[:, h * 2 + half, :],
                    start=(i_mm == 0), stop=(i_mm == n_mm - 1))
                i_mm += 1
        out_sb = consts.tile([BC, V], F32, tag="out_sb")
        nc.vector.tensor_add(out_sb, po[:BC, :], ba_rep)
        nc.sync.dma_start(out=t_out, in_=out_sb)


_NC_CACHE = None


def _get_nc():
    global _NC_CACHE
    if _NC_CACHE is None:
        _NC_CACHE = build_nc()
    return _NC_CACHE


def make_in_maps(query, keys, vals, rpe_mod, Wq, bq, Wa, ba, steps):
    wq16 = np.ascontiguousarray(Wq, dtype=np.float16)
    wa16 = np.ascontiguousarray(Wa, dtype=np.float16)
    bq32 = np.ascontiguousarray(bq, dtype=np.float32)
    ba32 = np.ascontiguousarray(ba, dtype=np.float32)
    rpe = np.asarray(rpe_mod)[:, :, 0]  # [L, B]
    in_maps = []
    for c in range(NCORES):
        bs = slice(c * BC, (c + 1) * BC)
        # keysT[ch, kp, kc, b, l] = keys[ch*256 + l, b, kc*128 + kp]
        kc_ = np.asarray(keys[:, bs, :]).reshape(NKC, KCL, BC, 2, 128)
        keysT = np.ascontiguousarray(
            kc_.transpose(0, 4, 3, 2, 1), dtype=np.float16)
        vals_c = np.ascontiguousarray(
            np.asarray(vals[:, bs, :]).reshape(NVC, 128, BC, V),
            dtype=np.float16)
        rpeT = np.ascontiguousarray(
            np.repeat(rpe[:, bs].T, H, axis=0), dtype=np.float16)
        stepsf = np.repeat(
            np.asarray(steps[bs]).astype(np.float32), H)
        in_maps.append({
            "query": np.ascontiguousarray(query[bs], dtype=np.float32),
            "keysT": keysT,
            "vals": vals_c,
            "rpeT": rpeT,
            "wq": wq16,
            "bq": bq32,
            "wa": wa16,
            "ba": ba32,
            "stepsf": np.ascontiguousarray(stepsf, dtype=np.float32),
        })
    return in_maps


def kernel(query, keys, vals, rpe_mod, Wq, bq, Wa, ba, steps):
    query = np.asarray(query)
    keys = np.asarray(keys)
    vals = np.asarray(vals)
    rpe_mod = np.asarray(rpe_mod)
    Wq = np.asarray(Wq)
    bq = np.asarray(bq)
    Wa = np.asarray(Wa)
    ba = np.asarray(ba)
    steps = np.asarray(steps)

    nc = _get_nc()
    in_maps = make_in_maps(query, keys, vals, rpe_mod, Wq, bq, Wa, ba, steps)
    res = run_bass_kernel_spmd(nc, in_maps, core_ids=list(range(NCORES)))
    out = np.concatenate([r["out"] for r in res.results], axis=0)
    return out.astype(np.float32)


# revision 17
# speedup vs baseline: 2.6698x; 1.3229x over previous
"""Trainium2 Bass kernel for the DND retrieval problem.

Full (unsharded) inputs in, full output out. Data-parallel over batch B=64
across 8 NeuronCores (8 batch elements per core), one SPMD Bass program.

Design notes:
- Every large operand ships as fp16 (half the HBM bytes of f32; fp16's
  10-bit mantissa keeps absmax rel err ~3e-3, well under the 2e-2 gate).
- keys are pre-transposed on the host to [k, l] layout so no on-chip
  transposes are needed for the scores matmuls.
- All tensors are SBUF-resident, so every DMA is issued eagerly with no
  waits and the DMA engines stream back-to-back.
- Softmax weights are exactly zero for l >= steps[b], so those (l, b)
  slices of keys/vals are never needed. The host sorts batches by steps
  (descending) and deals them round-robin across cores, so per l-chunk the
  live batches are a prefix and all cores share one live-count profile;
  the program is specialized to that profile (cached per profile) and only
  streams/computes the live prefix of each chunk (~35% fewer bytes for
  uniform steps).

Per-core program (Bc=8, H=8, K=V=256, L=2048):
  qTblk[k, 64]   block-diagonal q (col 8b+h) from wq/query (PE prologue)
  scoresT[bh, l] 2*nb fp16 matmuls per 256-l chunk into a [64,2048] PSUM tile
  softmax over l: scoresT = pscore*rpeT + addmask; global max; ACT Exp with
                  accum rowsums; recip folded back into scoresT
  readT[v, bh]   per 128-l chunk: PE transpose of w + 2*nb tiny (N=8)
                  matmuls accumulating in PSUM across all chunks
  out[b, :]      readT.T @ Wa + ba  (un-permuted on the host)
"""

import numpy as np

import concourse.bacc as bacc
import concourse.bass as bass
import concourse.mybir as mybir
import concourse.tile as tile
from concourse.bass_utils import run_bass_kernel_spmd
from concourse.masks import make_identity

F32 = mybir.dt.float32
F16 = mybir.dt.float16

L = 2048
B = 64
K = 256
V = 256
H = 8
NCORES = 8
BC = B // NCORES          # 8 batch elements per core
NKC = 8                   # keys chunks (256 l each)
KCL = L // NKC            # 256
NVC = 16                  # vals chunks (128 l each)
VCL = L // NVC            # 128
NEG = -1.0e30


def _ap(tensor_ap, offset_elems, dims):
    """Build a raw AP on the same tensor with explicit [step, count] dims."""
    return bass.AP(tensor=tensor_ap.tensor, offset=offset_elems, ap=dims)


def _plan(steps):
    """Sort batches by steps desc, deal round-robin to cores; live-count
    profiles per chunk (max across cores, so one SPMD program fits all)."""
    steps = np.asarray(steps)
    perm = np.argsort(-steps, kind="stable")
    core_idx = [perm[np.arange(BC) * NCORES + c] for c in range(NCORES)]
    nbk = [0] * NKC
    nbv = [0] * NVC
    for c in range(NCORES):
        sc = steps[core_idx[c]]
        for ch in range(NKC):
            nbk[ch] = max(nbk[ch], int((sc > ch * KCL).sum()))
        for vc in range(NVC):
            nbv[vc] = max(nbv[vc], int((sc > vc * VCL).sum()))
    return core_idx, tuple(nbk), tuple(nbv)


def build_nc(nbk, nbv):
    nc = bacc.Bacc("TRN2", target_bir_lowering=False)

    t_query = nc.dram_tensor("query", [BC, K], F32, kind="ExternalInput").ap()
    t_keysT = nc.dram_tensor("keysT", [NKC, 128, 2, BC, KCL], F16,
                             kind="ExternalInput").ap()
    t_vals = nc.dram_tensor("vals", [NVC, VCL, BC, V], F16,
                            kind="ExternalInput").ap()
    t_rpeT = nc.dram_tensor("rpeT", [B, L], F16, kind="ExternalInput").ap()
    t_wq = nc.dram_tensor("wq", [K, H * K], F16, kind="ExternalInput").ap()
    t_bq = nc.dram_tensor("bq", [H * K], F32, kind="ExternalInput").ap()
    t_wa = nc.dram_tensor("wa", [H * V, V], F16, kind="ExternalInput").ap()
    t_ba = nc.dram_tensor("ba", [V], F32, kind="ExternalInput").ap()
    t_steps = nc.dram_tensor("stepsf", [B], F32, kind="ExternalInput").ap()
    t_out = nc.dram_tensor("out", [BC, V], F32, kind="ExternalOutput").ap()

    with tile.TileContext(nc) as tc:
        _emit(nc, tc, t_query, t_keysT, t_vals, t_rpeT, t_wq, t_bq, t_wa,
              t_ba, t_steps, t_out, nbk, nbv)
    nc.compile()
    return nc


def _emit(nc, tc, t_query, t_keysT, t_vals, t_rpeT, t_wq, t_bq, t_wa, t_ba,
          t_steps, t_out, nbk, nbv):
    from contextlib import ExitStack
    ctx = ExitStack()
    with ctx:
        consts = ctx.enter_context(tc.tile_pool(name="consts", bufs=1))
        keysP = ctx.enter_context(tc.tile_pool(name="keysP", bufs=NKC))
        valsP = ctx.enter_context(tc.tile_pool(name="valsP", bufs=NVC))
        wsbP = ctx.enter_context(tc.tile_pool(name="wsbP", bufs=NVC))
        # PSUM budget is 8 banks: bigP (1) is time-shared by the prologue
        # transposes/q-build and po; pscP (2) double-buffers per-chunk score
        # accumulators so chunk N+1's matmuls don't wait on chunk N's DVE
        # copyback; pwP (3) pipelines the w transposes; prP (2) holds the two
        # readT accumulators (one bank per vh half — interleaved accumulation
        # groups must not share a bank).
        bigP = ctx.enter_context(tc.tile_pool(name="bigP", bufs=1, space="PSUM"))
        pscP = ctx.enter_context(tc.tile_pool(name="pscP", bufs=2, space="PSUM"))
        pwP = ctx.enter_context(tc.tile_pool(name="pwP", bufs=3, space="PSUM"))
        prP = ctx.enter_context(tc.tile_pool(name="prP", bufs=2, space="PSUM"))

        # ------------- DMA issue: weights first, then keys, then vals -------
        # Big stream on the SP/HWDGE queue, in order; side loads on the
        # Pool/SWDGE queue. All tiles are persistent, so no DMA ever waits.
        # Only the live batch prefix of each chunk is streamed.
        wmat = consts.tile([128, 2, H * K], F16, tag="wmat")
        nc.sync.dma_start(out=wmat, in_=t_wq.rearrange("(a p) j -> p a j", a=2))

        keys_tiles = []
        for ch in range(NKC):
            nb = nbk[ch]
            if nb == 0:
                keys_tiles.append(None)
                continue
            kt = keysP.tile([128, 2, nb, KCL], F16, tag="keys")
            nc.sync.dma_start(out=kt, in_=t_keysT[ch][:, :, :nb, :])
            keys_tiles.append(kt)

        vals_tiles = []
        for vc in range(NVC):
            nb = nbv[vc]
            if nb == 0:
                vals_tiles.append(None)
                continue
            vt = valsP.tile([VCL, nb, V], F16, tag="vals")
            nc.sync.dma_start(out=vt, in_=t_vals[vc][:, :nb, :])
            vals_tiles.append(vt)

        query_sb = consts.tile([BC, K], F32, tag="query")
        nc.gpsimd.dma_start(out=query_sb, in_=t_query)
        bq_nat = consts.tile([16, 128], F32, tag="bq_nat")
        nc.gpsimd.dma_start(out=bq_nat, in_=t_bq.rearrange("(r q) -> r q", r=16))
        stepsf = consts.tile([64, 1], F32, tag="stepsf")
        nc.gpsimd.dma_start(out=stepsf, in_=_ap(t_steps, 0, [[1, 64], [0, 1]]))
        rpeT = consts.tile([64, L], F16, tag="rpeT")
        nc.gpsimd.dma_start(out=rpeT, in_=t_rpeT)
        ba_rep = consts.tile([BC, V], F32, tag="ba_rep")
        nc.gpsimd.dma_start(out=ba_rep, in_=_ap(t_ba, 0, [[0, BC], [1, V]]))
        wa_sb = consts.tile([128, 16, V], F16, tag="wa_sb")
        nc.gpsimd.dma_start(
            out=wa_sb, in_=t_wa.rearrange("(a p) j -> p a j", a=16))

        # ------------- prologue compute -------------
        ident = consts.tile([128, 128], F32, tag="ident")
        make_identity(nc, ident)

        iota = consts.tile([64, L], F32, tag="iota")
        nc.gpsimd.iota(iota, pattern=[[1, L]], base=0, channel_multiplier=0,
                       allow_small_or_imprecise_dtypes=True)

        # scoresT starts at NEG: chunks/rows beyond the live prefix are never
        # written by the mults below and must read as fully-masked scores
        scoresT = consts.tile([64, L], F32, tag="scoresT")
        nc.vector.memset(scoresT, NEG)

        # queryT [k, b] (fp16) via PE transpose of query [b, k]
        queryT = consts.tile([128, 2, BC], F16, tag="queryT")
        for half in range(2):
            pq = bigP.tile([128, 256], F32, tag="big")
            nc.tensor.transpose(
                pq[:, :BC], query_sb[:, half * 128:(half + 1) * 128],
                ident[:BC, :BC])
            nc.any.tensor_copy(queryT[:, half, :], pq[:, :BC])

        # bqT [kout, (h,kc)] via PE transpose
        bq_sb = consts.tile([128, 16], F32, tag="bq_sb")
        pb = bigP.tile([128, 256], F32, tag="big")
        nc.tensor.transpose(pb[:, :16], bq_nat, ident[:16, :16])
        nc.any.tensor_copy(bq_sb, pb[:, :16])

        # block-diagonal qT: [kout(128), kc, b, 64 cols]; col 8b+h holds
        # q[b,h,kout], other columns zero, so one matmul per (kc, b)
        # accumulates all 64 (b,h) score rows without cross-terms
        qTblk = consts.tile([128, 2, BC, 64], F16, tag="qTblk")
        nc.vector.memset(qTblk, 0.0)
        # all 16 q matmuls into one PSUM tile first, then all scatters: no
        # per-(kc,h) PE<->ACT ping-pong on a shared buffer
        pq2 = bigP.tile([128, 16, BC], F32, tag="big", name="pq2")
        for kc in range(2):
            for h in range(H):
                idx = kc * H + h
                for kin in range(2):
                    col0 = h * K + kc * 128
                    nc.tensor.matmul(
                        pq2[:, idx, :],
                        lhsT=wmat[:, kin, col0:col0 + 128],
                        rhs=queryT[:, kin, :],
                        start=(kin == 0), stop=(kin == 1),
                        skip_group_check=True,
                    )
        for kc in range(2):
            for h in range(H):
                idx = kc * H + h
                # scatter b -> column 8b+h of batch-b's block (stride 72)
                out_ap = _ap(qTblk, kc * 512 + h,
                             [[qTblk.ap[0][0], 128], [72, BC]])
                if idx % 2 == 0:
                    nc.scalar.activation(
                        out_ap, pq2[:, idx, :],
                        mybir.ActivationFunctionType.Identity,
                        bias=bq_sb[:, h * 2 + kc:h * 2 + kc + 1], scale=1.0)
                else:
                    nc.vector.tensor_scalar(
                        out=out_ap, in0=pq2[:, idx, :],
                        scalar1=bq_sb[:, h * 2 + kc:h * 2 + kc + 1],
                        scalar2=None, op0=mybir.AluOpType.add)

        # additive -1e30 mask from runtime steps
        addmask = consts.tile([64, L], F32, tag="addmask")
        nc.vector.tensor_scalar(
            out=addmask, in0=iota, scalar1=stepsf, scalar2=NEG,
            op0=mybir.AluOpType.is_ge, op1=mybir.AluOpType.mult)

        # ------------- scores: one [64, 2048] PSUM tile -------------
        # Per chunk: 2*nb matmuls accumulate; rpe modulation is applied on
        # copyback of the live rows; the mask add is fused with a running
        # per-chunk max so everything trails the keys stream and negmax is
        # ready right after the last chunk.
        live_k = [ch for ch in range(NKC) if nbk[ch] > 0]
        runmax = consts.tile([64, NKC], F32, tag="runmax")
        for ch in live_k:
            kt = keys_tiles[ch]
            nb = nbk[ch]
            n_mm = 2 * nb
            i_mm = 0
            pscore = pscP.tile([64, KCL], F32, tag="psc", name="pscore")
            for kc in range(2):
                for b in range(nb):
                    nc.tensor.matmul(
                        pscore,
                        lhsT=qTblk[:, kc, b, :],
                        rhs=kt[:, kc, b, :],
                        start=(i_mm == 0), stop=(i_mm == n_mm - 1))
                    i_mm += 1
            lo = ch * KCL
            nc.vector.tensor_mul(scoresT[:8 * nb, lo:lo + KCL],
                                 pscore[:8 * nb, :],
                                 rpeT[:8 * nb, lo:lo + KCL])
            nc.vector.tensor_add(scoresT[:, lo:lo + KCL],
                                 scoresT[:, lo:lo + KCL],
                                 addmask[:, lo:lo + KCL])
            nc.vector.reduce_max(runmax[:, ch:ch + 1],
                                 scoresT[:, lo:lo + KCL],
                                 axis=mybir.AxisListType.X)

        # ------------- softmax over l (free dim) -------------
        # nbk is non-increasing (batches sorted by steps), so live chunks are
        # a prefix and runmax[:, :n_live] is exactly the written region
        negmax = consts.tile([64, 1], F32, tag="negmax")
        nc.vector.reduce_max(negmax, runmax[:, :len(live_k)],
                             axis=mybir.AxisListType.X, negate=True)
        psums = consts.tile([64, 4], F32, tag="psums")
        for c in range(4):
            lo = c * 512
            nc.scalar.activation(scoresT[:, lo:lo + 512],
                                 scoresT[:, lo:lo + 512],
                                 mybir.ActivationFunctionType.Exp,
                                 bias=negmax, scale=1.0,
                                 accum_out=psums[:, c:c + 1])
        sumexp = consts.tile([64, 1], F32, tag="sumexp")
        nc.vector.reduce_sum(sumexp, psums, axis=mybir.AxisListType.X)
        recip = consts.tile([64, 1], F32, tag="recip")
        nc.vector.reciprocal(recip, sumexp)
        # diag(recip): one regular matmul against it transposes a w chunk AND
        # applies the softmax denominator in the same PE pass
        dmat = consts.tile([64, 64], F32, tag="dmat")
        nc.vector.tensor_scalar(
            out=dmat, in0=ident[:64, :64], scalar1=recip, scalar2=None,
            op0=mybir.AluOpType.mult)

        # ------------- read: accumulate readT[v, bh] over all l -------------
        # lastvc[b]: the last chunk where batch-slot b is live (per-column
        # accumulation groups need their stop on their own final matmul).
        # The transpose+normalize matmuls (PE) are emitted one chunk ahead of
        # the read matmuls so PE never stalls on the w_sb copyback.
        lastvc = [max(vc for vc in range(NVC) if nbv[vc] > b)
                  for b in range(BC)]
        live_v = [vc for vc in range(NVC) if nbv[vc] > 0]
        preadT = [prP.tile([128, 64], F32, tag="pr", name=f"preadT{vh}")
                  for vh in range(2)]

        pw_tiles = {}

        def emit_wT(vc):
            pw = pwP.tile([128, 64], F32, tag="pw")
            off = vc * VCL
            nc.tensor.matmul(pw, lhsT=scoresT[:, off:off + VCL],
                             rhs=dmat, start=True, stop=True)
            pw_tiles[vc] = pw

        emit_wT(live_v[0])
        for i, vc in enumerate(live_v):
            if i + 1 < len(live_v):
                emit_wT(live_v[i + 1])
            vt = vals_tiles[vc]
            nb = nbv[vc]
            w_sb = wsbP.tile([128, 64], F16, tag="wsb")
            cb = nc.vector.tensor_copy if i % 2 == 0 else nc.scalar.copy
            cb(w_sb, pw_tiles.pop(vc))
            for vh in range(2):
                for b in range(nb):
                    nc.tensor.matmul(
                        preadT[vh][:, 8 * b:8 * b + 8],
                        lhsT=vt[:, b, vh * 128:(vh + 1) * 128],
                        rhs=w_sb[:, 8 * b:8 * b + 8],
                        start=(vc == live_v[0] and b == 0),
                        stop=(vc == lastvc[b]),
                        skip_group_check=True)

        # ------------- epilogue: head aggregation + store -------------
        readT_sb = consts.tile([128, 2, 64], F16, tag="readT_sb")
        nc.vector.tensor_copy(readT_sb[:, 0, :], preadT[0])
        nc.scalar.copy(readT_sb[:, 1, :], preadT[1])

        po = bigP.tile([64, V], F32, tag="big", name="po")
        n_mm = 2 * H
        i_mm = 0
        for half in range(2):
            for h in range(H):
                lhsT = _ap(readT_sb, half * 64 + h,
                           [[readT_sb.ap[0][0], 128], [8, BC]])
                nc.tensor.matmul(
                    po[:BC, :], lhsT=lhsT, rhs=wa_sb[:, h * 2 + half, :],
                    start=(i_mm == 0), stop=(i_mm == n_mm - 1))
                i_mm += 1
        out_sb = consts.tile([BC, V], F32, tag="out_sb")
        nc.vector.tensor_add(out_sb, po[:BC, :], ba_rep)
        nc.sync.dma_start(out=t_out, in_=out_sb)


_NC_CACHE = {}
_LAST_NC = None


def _get_nc(nbk=None, nbv=None):
    global _LAST_NC
    if nbk is None:
        # test/profiling convenience: the program from the latest kernel()
        # call (or the untruncated profile if none was made yet)
        if _LAST_NC is None:
            return _get_nc((BC,) * NKC, (BC,) * NVC)
        return _LAST_NC
    key = (nbk, nbv)
    if key not in _NC_CACHE:
        _NC_CACHE[key] = build_nc(nbk, nbv)
    _LAST_NC = _NC_CACHE[key]
    return _LAST_NC


def make_in_maps(query, keys, vals, rpe_mod, Wq, bq, Wa, ba, steps):
    core_idx, _, _ = _plan(steps)
    wq16 = np.ascontiguousarray(Wq, dtype=np.float16)
    wa16 = np.ascontiguousarray(Wa, dtype=np.float16)
    bq32 = np.ascontiguousarray(bq, dtype=np.float32)
    ba32 = np.ascontiguousarray(ba, dtype=np.float32)
    rpe = np.asarray(rpe_mod)[:, :, 0]  # [L, B]
    in_maps = []
    for c in range(NCORES):
        bs = core_idx[c]
        # keysT[ch, kp, kc, b, l] = keys[ch*256 + l, b, kc*128 + kp]
        kc_ = np.asarray(keys[:, bs, :]).reshape(NKC, KCL, BC, 2, 128)
        keysT = np.ascontiguousarray(
            kc_.transpose(0, 4, 3, 2, 1), dtype=np.float16)
        vals_c = np.ascontiguousarray(
            np.asarray(vals[:, bs, :]).reshape(NVC, VCL, BC, V),
            dtype=np.float16)
        rpeT = np.ascontiguousarray(
            np.repeat(rpe[:, bs].T, H, axis=0), dtype=np.float16)
        stepsf = np.repeat(
            np.asarray(steps[bs]).astype(np.float32), H)
        in_maps.append({
            "query": np.ascontiguousarray(query[bs], dtype=np.float32),
            "keysT": keysT,
            "vals": vals_c,
            "rpeT": rpeT,
            "wq": wq16,
            "bq": bq32,
            "wa": wa16,
            "ba": ba32,
            "stepsf": np.ascontiguousarray(stepsf, dtype=np.float32),
        })
    return in_maps


def kernel(query, keys, vals, rpe_mod, Wq, bq, Wa, ba, steps):
    query = np.asarray(query)
    keys = np.asarray(keys)
    vals = np.asarray(vals)
    rpe_mod = np.asarray(rpe_mod)
    Wq = np.asarray(Wq)
    bq = np.asarray(bq)
    Wa = np.asarray(Wa)
    ba = np.asarray(ba)
    steps = np.asarray(steps)

    core_idx, nbk, nbv = _plan(steps)
    nc = _get_nc(nbk, nbv)
    in_maps = make_in_maps(query, keys, vals, rpe_mod, Wq, bq, Wa, ba, steps)
    res = run_bass_kernel_spmd(nc, in_maps, core_ids=list(range(NCORES)))
    out = np.empty((B, V), dtype=np.float32)
    for c in range(NCORES):
        out[core_idx[c]] = res.results[c]["out"].astype(np.float32)
    return out


# revision 22
# speedup vs baseline: 2.9081x; 1.0893x over previous
"""Trainium2 Bass kernel for the DND retrieval problem.

Full (unsharded) inputs in, full output out. Data-parallel over batch B=64
across 8 NeuronCores (8 batch elements per core), one SPMD Bass program.

Design notes:
- Every large operand ships as fp16 (half the HBM bytes of f32; fp16's
  10-bit mantissa keeps absmax rel err ~3e-3, well under the 2e-2 gate).
- keys are pre-transposed on the host to [k, l] layout so no on-chip
  transposes are needed for the scores matmuls.
- All tensors are SBUF-resident, so every DMA is issued eagerly with no
  waits and the DMA engines stream back-to-back.
- Softmax weights are exactly zero for l >= steps[b], so those (l, b)
  slices of keys/vals are never needed. The host sorts batches by steps
  (descending) and deals them round-robin across cores, so per l-chunk the
  live batches are a prefix and all cores share one live-count profile;
  the program is specialized to that profile (cached per profile) and only
  streams/computes the live prefix of each chunk (~35% fewer bytes for
  uniform steps).

Per-core program (Bc=8, H=8, K=V=256, L=2048):
  qTblk[k, 64]   block-diagonal q (col 8b+h) from wq/query (PE prologue)
  scoresT[bh, l] 2*nb fp16 matmuls per 256-l chunk into a [64,2048] PSUM tile
  softmax over l: scoresT = pscore*rpeT + addmask; global max; ACT Exp with
                  accum rowsums; recip folded back into scoresT
  readT[v, bh]   per 128-l chunk: PE transpose of w + 2*nb tiny (N=8)
                  matmuls accumulating in PSUM across all chunks
  out[b, :]      readT.T @ Wa + ba  (un-permuted on the host)
"""

import numpy as np

import concourse.bacc as bacc
import concourse.bass as bass
import concourse.mybir as mybir
import concourse.tile as tile
from concourse.bass_utils import run_bass_kernel_spmd
from concourse.masks import make_identity
F32 = mybir.dt.float32
F16 = mybir.dt.float16

L = 2048
B = 64
K = 256
V = 256
H = 8
NCORES = 8
BC = B // NCORES          # 8 batch elements per core
NKC = 8                   # keys chunks (256 l each)
KCL = L // NKC            # 256
NVC = 16                  # vals chunks (128 l each)
VCL = L // NVC            # 128
NEG = -1.0e30


def _ap(tensor_ap, offset_elems, dims):
    """Build a raw AP on the same tensor with explicit [step, count] dims."""
    return bass.AP(tensor=tensor_ap.tensor, offset=offset_elems, ap=dims)


def _plan(steps):
    """Sort batches by steps desc, deal round-robin to cores; live-count
    profiles per chunk (max across cores, so one SPMD program fits all)."""
    steps = np.asarray(steps)
    perm = np.argsort(-steps, kind="stable")
    core_idx = [perm[np.arange(BC) * NCORES + c] for c in range(NCORES)]
    nbk = [0] * NKC
    nbv = [0] * NVC
    for c in range(NCORES):
        sc = steps[core_idx[c]]
        for ch in range(NKC):
            nbk[ch] = max(nbk[ch], int((sc > ch * KCL).sum()))
        for vc in range(NVC):
            nbv[vc] = max(nbv[vc], int((sc > vc * VCL).sum()))
    return core_idx, tuple(nbk), tuple(nbv)


def build_nc(nbk, nbv):
    nc = bacc.Bacc("TRN2", target_bir_lowering=False)

    t_query = nc.dram_tensor("query", [BC, K], F32, kind="ExternalInput").ap()
    t_keysT = nc.dram_tensor("keysT", [NKC, 128, 2, BC, KCL], F16,
                             kind="ExternalInput").ap()
    t_vals = nc.dram_tensor("vals", [NVC, VCL, BC, V], F16,
                            kind="ExternalInput").ap()
    t_rpeT = nc.dram_tensor("rpeT", [B, L], F16, kind="ExternalInput").ap()
    t_wq = nc.dram_tensor("wq", [K, H * K], F16, kind="ExternalInput").ap()
    t_bq = nc.dram_tensor("bq", [H * K], F32, kind="ExternalInput").ap()
    t_wa = nc.dram_tensor("wa", [H * V, V], F16, kind="ExternalInput").ap()
    t_ba = nc.dram_tensor("ba", [V], F32, kind="ExternalInput").ap()
    t_steps = nc.dram_tensor("stepsf", [B], F32, kind="ExternalInput").ap()
    t_out = nc.dram_tensor("out", [BC, V], F32, kind="ExternalOutput").ap()

    with tile.TileContext(nc) as tc:
        _emit(nc, tc, t_query, t_keysT, t_vals, t_rpeT, t_wq, t_bq, t_wa,
              t_ba, t_steps, t_out, nbk, nbv)
    nc.compile()
    return nc


def _emit(nc, tc, t_query, t_keysT, t_vals, t_rpeT, t_wq, t_bq, t_wa, t_ba,
          t_steps, t_out, nbk, nbv):
    from contextlib import ExitStack
    ctx = ExitStack()
    with ctx:
        consts = ctx.enter_context(tc.tile_pool(name="consts", bufs=1))
        keysP = ctx.enter_context(tc.tile_pool(name="keysP", bufs=NKC))
        valsP = ctx.enter_context(tc.tile_pool(name="valsP", bufs=NVC))
        wsbP = ctx.enter_context(tc.tile_pool(name="wsbP", bufs=NVC))
        # PSUM budget is 8 banks: bigP (1) is time-shared by the prologue
        # transposes/q-build and po; pscP (2) double-buffers per-chunk score
        # accumulators so chunk N+1's matmuls don't wait on chunk N's DVE
        # copyback; pwP (3) pipelines the w transposes; prP (2) holds the two
        # readT accumulators (one bank per vh half — interleaved accumulation
        # groups must not share a bank).
        bigP = ctx.enter_context(tc.tile_pool(name="bigP", bufs=1, space="PSUM"))
        pscP = ctx.enter_context(tc.tile_pool(name="pscP", bufs=2, space="PSUM"))
        pwP = ctx.enter_context(tc.tile_pool(name="pwP", bufs=3, space="PSUM"))
        prP = ctx.enter_context(tc.tile_pool(name="prP", bufs=2, space="PSUM"))

        # ------------- DMA issue -------------
        # One deterministic stream on the SP/HWDGE queue (FIFO on the DMA
        # engines): wmat -> rpe -> keys -> ba/wa -> vals. The softmax chain
        # hangs off the LAST keys chunk, so keys go as early as possible;
        # ba/wa hide inside the 15us vals stream; total DMA time is fixed by
        # bytes, only the ordering of the tail matters. Tiny loads ride the
        # Activation HWDGE queue; Pool only builds ident/iota, so the PE
        # prologue is ready before the first keys chunk lands.
        wmat = consts.tile([128, 2, H * K], F16, tag="wmat")
        nc.sync.dma_start(out=wmat, in_=t_wq.rearrange("(a p) j -> p a j", a=2))
        rpeT = consts.tile([64, L], F16, tag="rpeT")
        nc.sync.dma_start(out=rpeT, in_=t_rpeT)

        keys_tiles = []
        for ch in range(NKC):
            nb = nbk[ch]
            if nb == 0:
                keys_tiles.append(None)
                continue
            kt = keysP.tile([128, 2, nb, KCL], F16, tag="keys")
            nc.sync.dma_start(out=kt, in_=t_keysT[ch][:, :, :nb, :])
            keys_tiles.append(kt)

        ba_rep = consts.tile([BC, V], F32, tag="ba_rep")
        nc.sync.dma_start(out=ba_rep, in_=_ap(t_ba, 0, [[0, BC], [1, V]]))
        wa_sb = consts.tile([128, 16, V], F16, tag="wa_sb")
        nc.sync.dma_start(
            out=wa_sb, in_=t_wa.rearrange("(a p) j -> p a j", a=16))

        vals_tiles = []
        for vc in range(NVC):
            nb = nbv[vc]
            if nb == 0:
                vals_tiles.append(None)
                continue
            vt = valsP.tile([VCL, nb, V], F16, tag="vals")
            nc.sync.dma_start(out=vt, in_=t_vals[vc][:, :nb, :])
            vals_tiles.append(vt)

        query_sb = consts.tile([BC, K], F32, tag="query")
        nc.scalar.dma_start(out=query_sb, in_=t_query)
        bq_nat = consts.tile([16, 128], F32, tag="bq_nat")
        nc.scalar.dma_start(out=bq_nat, in_=t_bq.rearrange("(r q) -> r q", r=16))
        stepsf = consts.tile([64, 1], F32, tag="stepsf")
        nc.scalar.dma_start(out=stepsf, in_=_ap(t_steps, 0, [[1, 64], [0, 1]]))
        ident = consts.tile([128, 128], F32, tag="ident")
        make_identity(nc, ident)
        iota = consts.tile([64, L], F32, tag="iota")
        nc.gpsimd.iota(iota, pattern=[[1, L]], base=0, channel_multiplier=0,
                       allow_small_or_imprecise_dtypes=True)
        # ------------- prologue compute -------------
        # scoresT starts at NEG: chunks/rows beyond the live prefix are never
        # written by the mults below and must read as fully-masked scores
        scoresT = consts.tile([64, L], F32, tag="scoresT")
        nc.vector.memset(scoresT, NEG)

        # queryT [k, b] (fp16) via PE transpose of query [b, k]
        queryT = consts.tile([128, 2, BC], F16, tag="queryT")
        for half in range(2):
            pq = bigP.tile([128, 256], F32, tag="big")
            nc.tensor.transpose(
                pq[:, :BC], query_sb[:, half * 128:(half + 1) * 128],
                ident[:BC, :BC])
            nc.any.tensor_copy(queryT[:, half, :], pq[:, :BC])

        # bqT [kout, (h,kc)] via PE transpose
        bq_sb = consts.tile([128, 16], F32, tag="bq_sb")
        pb = bigP.tile([128, 256], F32, tag="big")
        nc.tensor.transpose(pb[:, :16], bq_nat, ident[:16, :16])
        nc.any.tensor_copy(bq_sb, pb[:, :16])

        # block-diagonal qT: [kout(128), kc, b, 64 cols]; col 8b+h holds
        # q[b,h,kout], other columns zero, so one matmul per (kc, b)
        # accumulates all 64 (b,h) score rows without cross-terms
        qTblks = []
        for kc in range(2):
            qTblk = consts.tile([128, BC, 64], F16, tag=f"qTblk{kc}",
                                name=f"qTblk{kc}")
            nc.vector.memset(qTblk, 0.0)
            qTblks.append(qTblk)
        # all 16 q matmuls into one PSUM tile first, then all scatters: no
        # per-(kc,h) PE<->ACT ping-pong on a shared buffer
        pq2 = bigP.tile([128, 16, BC], F32, tag="big", name="pq2")
        for kc in range(2):
            for h in range(H):
                idx = kc * H + h
                for kin in range(2):
                    col0 = h * K + kc * 128
                    nc.tensor.matmul(
                        pq2[:, idx, :],
                        lhsT=wmat[:, kin, col0:col0 + 128],
                        rhs=queryT[:, kin, :],
                        start=(kin == 0), stop=(kin == 1),
                        skip_group_check=True,
                    )
        for kc in range(2):
            for h in range(H):
                idx = kc * H + h
                # scatter b -> column 8b+h of batch-b's block (stride 72)
                out_ap = _ap(qTblks[kc], h,
                             [[qTblks[kc].ap[0][0], 128], [72, BC]])
                if kc == 0:
                    nc.scalar.activation(
                        out_ap, pq2[:, idx, :],
                        mybir.ActivationFunctionType.Identity,
                        bias=bq_sb[:, h * 2 + kc:h * 2 + kc + 1], scale=1.0)
                else:
                    nc.vector.tensor_scalar(
                        out=out_ap, in0=pq2[:, idx, :],
                        scalar1=bq_sb[:, h * 2 + kc:h * 2 + kc + 1],
                        scalar2=None, op0=mybir.AluOpType.add)

        # additive -1e30 mask from runtime steps
        addmask = consts.tile([64, L], F32, tag="addmask")
        nc.vector.tensor_scalar(
            out=addmask, in0=iota, scalar1=stepsf, scalar2=NEG,
            op0=mybir.AluOpType.is_ge, op1=mybir.AluOpType.mult)

        # ------------- scores: one [64, 2048] PSUM tile -------------
        # Per chunk: 2*nb matmuls accumulate; rpe modulation is applied on
        # copyback of the live rows; the mask add is fused with a running
        # per-chunk max so everything trails the keys stream and negmax is
        # ready right after the last chunk.
        live_k = [ch for ch in range(NKC) if nbk[ch] > 0]
        runmax = consts.tile([64, NKC], F32, tag="runmax")
        for ch in live_k:
            kt = keys_tiles[ch]
            nb = nbk[ch]
            n_mm = 2 * nb
            i_mm = 0
            pscore = pscP.tile([64, KCL], F32, tag="psc", name="pscore")
            for kc in range(2):
                for b in range(nb):
                    nc.tensor.matmul(
                        pscore,
                        lhsT=qTblks[kc][:, b, :],
                        rhs=kt[:, kc, b, :],
                        start=(i_mm == 0), stop=(i_mm == n_mm - 1))
                    i_mm += 1
            lo = ch * KCL
            nc.vector.tensor_mul(scoresT[:8 * nb, lo:lo + KCL],
                                 pscore[:8 * nb, :],
                                 rpeT[:8 * nb, lo:lo + KCL])
            nc.vector.tensor_add(scoresT[:, lo:lo + KCL],
                                 scoresT[:, lo:lo + KCL],
                                 addmask[:, lo:lo + KCL])
            nc.vector.reduce_max(runmax[:, ch:ch + 1],
                                 scoresT[:, lo:lo + KCL],
                                 axis=mybir.AxisListType.X)

        # ------------- softmax over l (free dim) -------------
        # nbk is non-increasing (batches sorted by steps), so live chunks are
        # a prefix and runmax[:, :n_live] is exactly the written region
        negmax = consts.tile([64, 1], F32, tag="negmax")
        nc.vector.reduce_max(negmax, runmax[:, :len(live_k)],
                             axis=mybir.AxisListType.X, negate=True)
        psums = consts.tile([64, 4], F32, tag="psums")
        for c in range(4):
            lo = c * 512
            nc.scalar.activation(scoresT[:, lo:lo + 512],
                                 scoresT[:, lo:lo + 512],
                                 mybir.ActivationFunctionType.Exp,
                                 bias=negmax, scale=1.0,
                                 accum_out=psums[:, c:c + 1])
        sumexp = consts.tile([64, 1], F32, tag="sumexp")
        nc.vector.reduce_sum(sumexp, psums, axis=mybir.AxisListType.X)
        recip = consts.tile([64, 1], F32, tag="recip")
        nc.vector.reciprocal(recip, sumexp)
        # diag(recip): one regular matmul against it transposes a w chunk AND
        # applies the softmax denominator in the same PE pass (out[l, bh] =
        # sum_r scoresT[r, l] * diag[r, bh] = scoresT[bh, l] * recip[bh])
        dmat = consts.tile([64, 64], F32, tag="dmat")
        nc.vector.tensor_scalar(
            out=dmat, in0=ident[:64, :64], scalar1=recip, scalar2=None,
            op0=mybir.AluOpType.mult)

        # ------------- read: accumulate readT[v, bh] over all l -------------
        # lastvc[b]: the last chunk where batch-slot b is live (per-column
        # accumulation groups need their stop on their own final matmul).
        # The transpose+normalize matmuls (PE) are emitted one chunk ahead of
        # the read matmuls so PE never stalls on the w_sb copyback.
        lastvc = [max(vc for vc in range(NVC) if nbv[vc] > b)
                  for b in range(BC)]
        live_v = [vc for vc in range(NVC) if nbv[vc] > 0]
        preadT = [prP.tile([128, 64], F32, tag="pr", name=f"preadT{vh}")
                  for vh in range(2)]

        pw_tiles = {}

        def emit_wT(vc):
            pw = pwP.tile([128, 64], F32, tag="pw")
            off = vc * VCL
            nc.tensor.matmul(pw, lhsT=scoresT[:, off:off + VCL],
                             rhs=dmat, start=True, stop=True)
            pw_tiles[vc] = pw

        emit_wT(live_v[0])
        for i, vc in enumerate(live_v):
            if i + 1 < len(live_v):
                emit_wT(live_v[i + 1])
            vt = vals_tiles[vc]
            nb = nbv[vc]
            w_sb = wsbP.tile([128, 64], F16, tag="wsb")
            cb = nc.vector.tensor_copy if i % 2 == 0 else nc.scalar.copy
            cb(w_sb, pw_tiles.pop(vc))
            for vh in range(2):
                for b in range(nb):
                    nc.tensor.matmul(
                        preadT[vh][:, 8 * b:8 * b + 8],
                        lhsT=vt[:, b, vh * 128:(vh + 1) * 128],
                        rhs=w_sb[:, 8 * b:8 * b + 8],
                        start=(vc == live_v[0] and b == 0),
                        stop=(vc == lastvc[b]),
                        skip_group_check=True)

        # ------------- epilogue: head aggregation + store -------------
        readT_sb = consts.tile([128, 2, 64], F16, tag="readT_sb")
        nc.vector.tensor_copy(readT_sb[:, 0, :], preadT[0])
        nc.scalar.copy(readT_sb[:, 1, :], preadT[1])

        po = bigP.tile([64, V], F32, tag="big", name="po")
        n_mm = 2 * H
        i_mm = 0
        for half in range(2):
            for h in range(H):
                lhsT = _ap(readT_sb, half * 64 + h,
                           [[readT_sb.ap[0][0], 128], [8, BC]])
                nc.tensor.matmul(
                    po[:BC, :], lhsT=lhsT, rhs=wa_sb[:, h * 2 + half, :],
                    start=(i_mm == 0), stop=(i_mm == n_mm - 1))
                i_mm += 1
        out_sb = consts.tile([BC, V], F32, tag="out_sb")
        nc.vector.tensor_add(out_sb, po[:BC, :], ba_rep)
        nc.sync.dma_start(out=t_out, in_=out_sb)


_NC_CACHE = {}
_LAST_NC = None


def _get_nc(nbk=None, nbv=None):
    global _LAST_NC
    if nbk is None:
        # test/profiling convenience: the program from the latest kernel()
        # call (or the untruncated profile if none was made yet)
        if _LAST_NC is None:
            return _get_nc((BC,) * NKC, (BC,) * NVC)
        return _LAST_NC
    key = (nbk, nbv)
    if key not in _NC_CACHE:
        _NC_CACHE[key] = build_nc(nbk, nbv)
    _LAST_NC = _NC_CACHE[key]
    return _LAST_NC


def make_in_maps(query, keys, vals, rpe_mod, Wq, bq, Wa, ba, steps):
    core_idx, _, _ = _plan(steps)
    wq16 = np.ascontiguousarray(Wq, dtype=np.float16)
    wa16 = np.ascontiguousarray(Wa, dtype=np.float16)
    bq32 = np.ascontiguousarray(bq, dtype=np.float32)
    ba32 = np.ascontiguousarray(ba, dtype=np.float32)
    rpe = np.asarray(rpe_mod)[:, :, 0]  # [L, B]
    in_maps = []
    for c in range(NCORES):
        bs = core_idx[c]
        # keysT[ch, kp, kc, b, l] = keys[ch*256 + l, b, kc*128 + kp]
        kc_ = np.asarray(keys[:, bs, :]).reshape(NKC, KCL, BC, 2, 128)
        keysT = np.ascontiguousarray(
            kc_.transpose(0, 4, 3, 2, 1), dtype=np.float16)
        vals_c = np.ascontiguousarray(
            np.asarray(vals[:, bs, :]).reshape(NVC, VCL, BC, V),
            dtype=np.float16)
        rpeT = np.ascontiguousarray(
            np.repeat(rpe[:, bs].T, H, axis=0), dtype=np.float16)
        stepsf = np.repeat(
            np.asarray(steps[bs]).astype(np.float32), H)
        in_maps.append({
            "query": np.ascontiguousarray(query[bs], dtype=np.float32),
            "keysT": keysT,
            "vals": vals_c,
            "rpeT": rpeT,
            "wq": wq16,
            "bq": bq32,
            "wa": wa16,
            "ba": ba32,
            "stepsf": np.ascontiguousarray(stepsf, dtype=np.float32),
        })
    return in_maps


def kernel(query, keys, vals, rpe_mod, Wq, bq, Wa, ba, steps):
    query = np.asarray(query)
    keys = np.asarray(keys)
    vals = np.asarray(vals)
    rpe_mod = np.asarray(rpe_mod)
    Wq = np.asarray(Wq)
    bq = np.asarray(bq)
    Wa = np.asarray(Wa)
    ba = np.asarray(ba)
    steps = np.asarray(steps)

    core_idx, nbk, nbv = _plan(steps)
    nc = _get_nc(nbk, nbv)
    in_maps = make_in_maps(query, keys, vals, rpe_mod, Wq, bq, Wa, ba, steps)
    res = run_bass_kernel_spmd(nc, in_maps, core_ids=list(range(NCORES)))
    out = np.empty((B, V), dtype=np.float32)
    for c in range(NCORES):
        out[core_idx[c]] = res.results[c]["out"].astype(np.float32)
    return out


# revision 30
# speedup vs baseline: 2.9518x; 1.0150x over previous
"""Trainium2 Bass kernel for the DND retrieval problem.

Full (unsharded) inputs in, full output out. Data-parallel over batch B=64
across 8 NeuronCores (8 batch elements per core), one SPMD Bass program.

Design notes:
- Every large operand ships as fp16 (half the HBM bytes of f32; fp16's
  10-bit mantissa keeps absmax rel err ~3e-3, well under the 2e-2 gate).
- keys are pre-transposed on the host to [k, l] layout so no on-chip
  transposes are needed for the scores matmuls.
- All tensors are SBUF-resident, so every DMA is issued eagerly with no
  waits and the DMA engines stream back-to-back.
- Softmax weights are exactly zero for l >= steps[b], so those (l, b)
  slices of keys/vals are never needed. The host sorts batches by steps
  (descending) and deals them round-robin across cores, so per l-chunk the
  live batches are a prefix and all cores share one live-count profile;
  the program is specialized to that profile (cached per profile) and only
  streams/computes the live prefix of each chunk (~35% fewer bytes for
  uniform steps).

Per-core program (Bc=8, H=8, K=V=256, L=2048):
  qTblk[k, 64]   block-diagonal q (col 8b+h) from wq/query (PE prologue)
  scoresT[bh, l] 2*nb fp16 matmuls per 256-l chunk into a [64,2048] PSUM tile
  softmax over l: scoresT = pscore*rpeT + addmask; global max; ACT Exp with
                  accum rowsums; recip folded back into scoresT
  readT[v, bh]   per 128-l chunk: PE transpose of w + 2*nb tiny (N=8)
                  matmuls accumulating in PSUM across all chunks
  out[b, :]      readT.T @ Wa + ba  (un-permuted on the host)
"""

import numpy as np

import concourse.bacc as bacc
import concourse.bass as bass
import concourse.mybir as mybir
import concourse.tile as tile
from concourse.bass_utils import run_bass_kernel_spmd
from concourse.masks import make_identity
F32 = mybir.dt.float32
F16 = mybir.dt.float16

L = 2048
B = 64
K = 256
V = 256
H = 8
NCORES = 8
BC = B // NCORES          # 8 batch elements per core
NKC = 8                   # keys chunks (256 l each)
KCL = L // NKC            # 256
NVC = 16                  # vals chunks (128 l each)
VCL = L // NVC            # 128
NEG = -1.0e30


def _ap(tensor_ap, offset_elems, dims):
    """Build a raw AP on the same tensor with explicit [step, count] dims."""
    return bass.AP(tensor=tensor_ap.tensor, offset=offset_elems, ap=dims)


def _plan(steps):
    """Sort batches by steps desc, deal round-robin to cores; live-count
    profiles per chunk (max across cores, so one SPMD program fits all)."""
    steps = np.asarray(steps)
    perm = np.argsort(-steps, kind="stable")
    core_idx = [perm[np.arange(BC) * NCORES + c] for c in range(NCORES)]
    nbk = [0] * NKC
    nbv = [0] * NVC
    for c in range(NCORES):
        sc = steps[core_idx[c]]
        for ch in range(NKC):
            nbk[ch] = max(nbk[ch], int((sc > ch * KCL).sum()))
        for vc in range(NVC):
            nbv[vc] = max(nbv[vc], int((sc > vc * VCL).sum()))
    # per batch slot: number of live l-partitions in its LAST live chunk
    # (the rest of that chunk has zero weight and is never loaded/computed)
    rend = [VCL] * BC
    for b in range(BC):
        lvc = max(vc for vc in range(NVC) if nbv[vc] > b)
        r = 1
        for c in range(NCORES):
            s = int(steps[core_idx[c][b]])
            r = max(r, min(s - lvc * VCL, VCL))
        rend[b] = r
    return core_idx, tuple(nbk), tuple(nbv), tuple(rend)


def build_nc(nbk, nbv, rend):
    nc = bacc.Bacc("TRN2", target_bir_lowering=False)

    t_query = nc.dram_tensor("query", [BC, K], F32, kind="ExternalInput").ap()
    t_keysT = nc.dram_tensor("keysT", [NKC, 128, 2, BC, KCL], F16,
                             kind="ExternalInput").ap()
    t_vals = nc.dram_tensor("vals", [NVC, VCL, BC, V], F16,
                            kind="ExternalInput").ap()
    t_rpeT = nc.dram_tensor("rpeT", [B, L], F16, kind="ExternalInput").ap()
    t_wq = nc.dram_tensor("wq", [K, H * K], F16, kind="ExternalInput").ap()
    t_bq = nc.dram_tensor("bq", [H * K], F32, kind="ExternalInput").ap()
    t_wa = nc.dram_tensor("wa", [H * V, V], F16, kind="ExternalInput").ap()
    t_ba = nc.dram_tensor("ba", [V], F32, kind="ExternalInput").ap()
    t_steps = nc.dram_tensor("stepsf", [B], F32, kind="ExternalInput").ap()
    t_out = nc.dram_tensor("out", [BC, V], F32, kind="ExternalOutput").ap()

    with tile.TileContext(nc) as tc:
        _emit(nc, tc, t_query, t_keysT, t_vals, t_rpeT, t_wq, t_bq, t_wa,
              t_ba, t_steps, t_out, nbk, nbv, rend)
    nc.compile()
    return nc


def _emit(nc, tc, t_query, t_keysT, t_vals, t_rpeT, t_wq, t_bq, t_wa, t_ba,
          t_steps, t_out, nbk, nbv, rend):
    from contextlib import ExitStack
    ctx = ExitStack()
    with ctx:
        consts = ctx.enter_context(tc.tile_pool(name="consts", bufs=1))
        keysP = ctx.enter_context(tc.tile_pool(name="keysP", bufs=NKC))
        valsP = ctx.enter_context(tc.tile_pool(name="valsP", bufs=NVC))
        wsbP = ctx.enter_context(tc.tile_pool(name="wsbP", bufs=NVC))
        # PSUM budget is 8 banks: bigP (1) is time-shared by the prologue
        # transposes/q-build and po; pscP (2) double-buffers per-chunk score
        # accumulators so chunk N+1's matmuls don't wait on chunk N's DVE
        # copyback; pwP (3) pipelines the w transposes; prP (2) holds the two
        # readT accumulators (one bank per vh half — interleaved accumulation
        # groups must not share a bank).
        bigP = ctx.enter_context(tc.tile_pool(name="bigP", bufs=1, space="PSUM"))
        pscP = ctx.enter_context(tc.tile_pool(name="pscP", bufs=2, space="PSUM"))
        pwP = ctx.enter_context(tc.tile_pool(name="pwP", bufs=3, space="PSUM"))
        prP = ctx.enter_context(tc.tile_pool(name="prP", bufs=2, space="PSUM"))

        # ------------- DMA issue -------------
        # One deterministic stream on the SP/HWDGE queue (FIFO on the DMA
        # engines): wmat -> rpe -> keys -> ba/wa -> vals. The softmax chain
        # hangs off the LAST keys chunk, so keys go as early as possible;
        # ba/wa hide inside the 15us vals stream; total DMA time is fixed by
        # bytes, only the ordering of the tail matters. Tiny loads ride the
        # Activation HWDGE queue; Pool only builds ident/iota, so the PE
        # prologue is ready before the first keys chunk lands.
        wmat = consts.tile([128, 2, H * K], F16, tag="wmat")
        nc.sync.dma_start(out=wmat, in_=t_wq.rearrange("(a p) j -> p a j", a=2))
        rpeT = consts.tile([64, L], F16, tag="rpeT")
        nc.sync.dma_start(out=rpeT, in_=t_rpeT)

        keys_tiles = []
        for ch in range(NKC):
            nb = nbk[ch]
            if nb == 0:
                keys_tiles.append(None)
                continue
            kt = keysP.tile([128, 2, nb, KCL], F16, tag="keys")
            nc.sync.dma_start(out=kt, in_=t_keysT[ch][:, :, :nb, :])
            keys_tiles.append(kt)

        ba_rep = consts.tile([BC, V], F32, tag="ba_rep")
        nc.sync.dma_start(out=ba_rep, in_=_ap(t_ba, 0, [[0, BC], [1, V]]))
        wa_sb = consts.tile([128, 16, V], F16, tag="wa_sb")
        nc.sync.dma_start(
            out=wa_sb, in_=t_wa.rearrange("(a p) j -> p a j", a=16))

        nbv_next = list(nbv[1:]) + [0]
        vals_tiles = []
        for vc in range(NVC):
            nb = nbv[vc]
            if nb == 0:
                vals_tiles.append(None)
                continue
            nb_full = nbv_next[vc]
            vt = valsP.tile([VCL, nb, V], F16, tag="vals")
            if nb_full > 0:
                nc.sync.dma_start(out=vt[:, :nb_full, :],
                                  in_=t_vals[vc][:, :nb_full, :])
            for b in range(nb_full, nb):
                # partial chunks ride the ACT queue: the SP queue's serial
                # issue rate would otherwise starve the stream tail
                r = rend[b]
                nc.gpsimd.dma_start(out=vt[:r, b:b + 1, :],
                                     in_=t_vals[vc][:r, b:b + 1, :])
            vals_tiles.append(vt)

        query_sb = consts.tile([BC, K], F32, tag="query")
        nc.scalar.dma_start(out=query_sb, in_=t_query)
        bq_nat = consts.tile([16, 128], F32, tag="bq_nat")
        nc.scalar.dma_start(out=bq_nat, in_=t_bq.rearrange("(r q) -> r q", r=16))
        stepsf = consts.tile([64, 1], F32, tag="stepsf")
        nc.scalar.dma_start(out=stepsf, in_=_ap(t_steps, 0, [[1, 64], [0, 1]]))
        ident = consts.tile([128, 128], F32, tag="ident")
        make_identity(nc, ident)
        iota = consts.tile([64, L], F32, tag="iota")
        nc.gpsimd.iota(iota, pattern=[[1, L]], base=0, channel_multiplier=0,
                       allow_small_or_imprecise_dtypes=True)
        # ------------- prologue compute -------------
        # scoresT starts at NEG: chunks/rows beyond the live prefix are never
        # written by the mults below and must read as fully-masked scores
        scoresT = consts.tile([64, L], F32, tag="scoresT")
        nc.vector.memset(scoresT, NEG)

        # queryT [k, b] (fp16) via PE transpose of query [b, k]
        queryT = consts.tile([128, 2, BC], F16, tag="queryT")
        for half in range(2):
            pq = bigP.tile([128, 256], F32, tag="big")
            nc.tensor.transpose(
                pq[:, :BC], query_sb[:, half * 128:(half + 1) * 128],
                ident[:BC, :BC])
            nc.any.tensor_copy(queryT[:, half, :], pq[:, :BC])

        # bqT [kout, (h,kc)] via PE transpose
        bq_sb = consts.tile([128, 16], F32, tag="bq_sb")
        pb = bigP.tile([128, 256], F32, tag="big")
        nc.tensor.transpose(pb[:, :16], bq_nat, ident[:16, :16])
        nc.any.tensor_copy(bq_sb, pb[:, :16])

        # block-diagonal qT: [kout(128), kc, b, 64 cols]; col 8b+h holds
        # q[b,h,kout], other columns zero, so one matmul per (kc, b)
        # accumulates all 64 (b,h) score rows without cross-terms
        qTblks = []
        for kc in range(2):
            qTblk = consts.tile([128, BC, 64], F16, tag=f"qTblk{kc}",
                                name=f"qTblk{kc}")
            nc.vector.memset(qTblk, 0.0)
            qTblks.append(qTblk)
        # all 16 q matmuls into one PSUM tile first, then all scatters: no
        # per-(kc,h) PE<->ACT ping-pong on a shared buffer
        pq2 = bigP.tile([128, 16, BC], F32, tag="big", name="pq2")
        for kc in range(2):
            for h in range(H):
                idx = kc * H + h
                for kin in range(2):
                    col0 = h * K + kc * 128
                    nc.tensor.matmul(
                        pq2[:, idx, :],
                        lhsT=wmat[:, kin, col0:col0 + 128],
                        rhs=queryT[:, kin, :],
                        start=(kin == 0), stop=(kin == 1),
                        skip_group_check=True,
                    )
        for kc in range(2):
            for h in range(H):
                idx = kc * H + h
                # scatter b -> column 8b+h of batch-b's block (stride 72)
                out_ap = _ap(qTblks[kc], h,
                             [[qTblks[kc].ap[0][0], 128], [72, BC]])
                if kc == 0:
                    nc.scalar.activation(
                        out_ap, pq2[:, idx, :],
                        mybir.ActivationFunctionType.Identity,
                        bias=bq_sb[:, h * 2 + kc:h * 2 + kc + 1], scale=1.0)
                else:
                    nc.vector.tensor_scalar(
                        out=out_ap, in0=pq2[:, idx, :],
                        scalar1=bq_sb[:, h * 2 + kc:h * 2 + kc + 1],
                        scalar2=None, op0=mybir.AluOpType.add)

        # additive -1e30 mask from runtime steps
        addmask = consts.tile([64, L], F32, tag="addmask")
        nc.vector.tensor_scalar(
            out=addmask, in0=iota, scalar1=stepsf, scalar2=NEG,
            op0=mybir.AluOpType.is_ge, op1=mybir.AluOpType.mult)

        # ------------- scores: one [64, 2048] PSUM tile -------------
        # Per chunk: 2*nb matmuls accumulate; rpe modulation is applied on
        # copyback of the live rows; the mask add is fused with a running
        # per-chunk max so everything trails the keys stream and negmax is
        # ready right after the last chunk.
        live_k = [ch for ch in range(NKC) if nbk[ch] > 0]
        runmax = consts.tile([64, NKC], F32, tag="runmax")
        for ch in live_k:
            kt = keys_tiles[ch]
            nb = nbk[ch]
            n_mm = 2 * nb
            i_mm = 0
            pscore = pscP.tile([64, KCL], F32, tag="psc", name="pscore")
            for kc in range(2):
                for b in range(nb):
                    nc.tensor.matmul(
                        pscore,
                        lhsT=qTblks[kc][:, b, :],
                        rhs=kt[:, kc, b, :],
                        start=(i_mm == 0), stop=(i_mm == n_mm - 1))
                    i_mm += 1
            lo = ch * KCL
            nc.vector.tensor_mul(scoresT[:8 * nb, lo:lo + KCL],
                                 pscore[:8 * nb, :],
                                 rpeT[:8 * nb, lo:lo + KCL])
            nc.vector.tensor_add(scoresT[:, lo:lo + KCL],
                                 scoresT[:, lo:lo + KCL],
                                 addmask[:, lo:lo + KCL])
            nc.vector.reduce_max(runmax[:, ch:ch + 1],
                                 scoresT[:, lo:lo + KCL],
                                 axis=mybir.AxisListType.X)

        # ------------- softmax over l (free dim) -------------
        # nbk is non-increasing (batches sorted by steps), so live chunks are
        # a prefix and runmax[:, :n_live] is exactly the written region
        negmax = consts.tile([64, 1], F32, tag="negmax")
        nc.vector.reduce_max(negmax, runmax[:, :len(live_k)],
                             axis=mybir.AxisListType.X, negate=True)
        psums = consts.tile([64, 4], F32, tag="psums")
        for c in range(4):
            lo = c * 512
            nc.scalar.activation(scoresT[:, lo:lo + 512],
                                 scoresT[:, lo:lo + 512],
                                 mybir.ActivationFunctionType.Exp,
                                 bias=negmax, scale=1.0,
                                 accum_out=psums[:, c:c + 1])
        sumexp = consts.tile([64, 1], F32, tag="sumexp")
        nc.vector.reduce_sum(sumexp, psums, axis=mybir.AxisListType.X)
        recip = consts.tile([64, 1], F32, tag="recip")
        nc.vector.reciprocal(recip, sumexp)
        # diag(recip): one regular matmul against it transposes a w chunk AND
        # applies the softmax denominator in the same PE pass (out[l, bh] =
        # sum_r scoresT[r, l] * diag[r, bh] = scoresT[bh, l] * recip[bh])
        dmat = consts.tile([64, 64], F32, tag="dmat")
        nc.vector.tensor_scalar(
            out=dmat, in0=ident[:64, :64], scalar1=recip, scalar2=None,
            op0=mybir.AluOpType.mult)

        # ------------- read: accumulate readT[v, bh] over all l -------------
        # lastvc[b]: the last chunk where batch-slot b is live (per-column
        # accumulation groups need their stop on their own final matmul).
        # The transpose+normalize matmuls (PE) are emitted one chunk ahead of
        # the read matmuls so PE never stalls on the w_sb copyback.
        lastvc = [max(vc for vc in range(NVC) if nbv[vc] > b)
                  for b in range(BC)]
        live_v = [vc for vc in range(NVC) if nbv[vc] > 0]
        preadT = [prP.tile([128, 64], F32, tag="pr", name=f"preadT{vh}")
                  for vh in range(2)]

        pw_tiles = {}

        def emit_wT(vc):
            pw = pwP.tile([128, 64], F32, tag="pw")
            off = vc * VCL
            nc.tensor.matmul(pw, lhsT=scoresT[:, off:off + VCL],
                             rhs=dmat, start=True, stop=True)
            pw_tiles[vc] = pw

        emit_wT(live_v[0])
        for i, vc in enumerate(live_v):
            if i + 1 < len(live_v):
                emit_wT(live_v[i + 1])
            vt = vals_tiles[vc]
            nb = nbv[vc]
            w_sb = wsbP.tile([128, 64], F16, tag="wsb")
            cb = nc.vector.tensor_copy if i % 2 == 0 else nc.scalar.copy
            cb(w_sb, pw_tiles.pop(vc))
            for vh in range(2):
                for b in range(nb):
                    r = rend[b] if vc == lastvc[b] else VCL
                    nc.tensor.matmul(
                        preadT[vh][:, 8 * b:8 * b + 8],
                        lhsT=vt[:r, b, vh * 128:(vh + 1) * 128],
                        rhs=w_sb[:r, 8 * b:8 * b + 8],
                        start=(vc == live_v[0] and b == 0),
                        stop=(vc == lastvc[b]),
                        skip_group_check=True)

        # ------------- epilogue: head aggregation + store -------------
        readT_sb = consts.tile([128, 2, 64], F16, tag="readT_sb")
        nc.vector.tensor_copy(readT_sb[:, 0, :], preadT[0])
        nc.scalar.copy(readT_sb[:, 1, :], preadT[1])

        po = bigP.tile([64, V], F32, tag="big", name="po")
        n_mm = 2 * H
        i_mm = 0
        for half in range(2):
            for h in range(H):
                lhsT = _ap(readT_sb, half * 64 + h,
                           [[readT_sb.ap[0][0], 128], [8, BC]])
                nc.tensor.matmul(
                    po[:BC, :], lhsT=lhsT, rhs=wa_sb[:, h * 2 + half, :],
                    start=(i_mm == 0), stop=(i_mm == n_mm - 1))
                i_mm += 1
        out_sb = consts.tile([BC, V], F32, tag="out_sb")
        nc.vector.tensor_add(out_sb, po[:BC, :], ba_rep)
        nc.sync.dma_start(out=t_out, in_=out_sb)


_NC_CACHE = {}
_LAST_NC = None


def _get_nc(nbk=None, nbv=None, rend=None):
    global _LAST_NC
    if nbk is None:
        # test/profiling convenience: the program from the latest kernel()
        # call (or the untruncated profile if none was made yet)
        if _LAST_NC is None:
            return _get_nc((BC,) * NKC, (BC,) * NVC, (VCL,) * BC)
        return _LAST_NC
    key = (nbk, nbv, rend)
    if key not in _NC_CACHE:
        _NC_CACHE[key] = build_nc(nbk, nbv, rend)
    _LAST_NC = _NC_CACHE[key]
    return _LAST_NC


def make_in_maps(query, keys, vals, rpe_mod, Wq, bq, Wa, ba, steps):
    core_idx = _plan(steps)[0]
    wq16 = np.ascontiguousarray(Wq, dtype=np.float16)
    wa16 = np.ascontiguousarray(Wa, dtype=np.float16)
    bq32 = np.ascontiguousarray(bq, dtype=np.float32)
    ba32 = np.ascontiguousarray(ba, dtype=np.float32)
    rpe = np.asarray(rpe_mod)[:, :, 0]  # [L, B]
    in_maps = []
    for c in range(NCORES):
        bs = core_idx[c]
        # keysT[ch, kp, kc, b, l] = keys[ch*256 + l, b, kc*128 + kp]
        kc_ = np.asarray(keys[:, bs, :]).reshape(NKC, KCL, BC, 2, 128)
        keysT = np.ascontiguousarray(
            kc_.transpose(0, 4, 3, 2, 1), dtype=np.float16)
        vals_c = np.ascontiguousarray(
            np.asarray(vals[:, bs, :]).reshape(NVC, VCL, BC, V),
            dtype=np.float16)
        rpeT = np.ascontiguousarray(
            np.repeat(rpe[:, bs].T, H, axis=0), dtype=np.float16)
        stepsf = np.repeat(
            np.asarray(steps[bs]).astype(np.float32), H)
        in_maps.append({
            "query": np.ascontiguousarray(query[bs], dtype=np.float32),
            "keysT": keysT,
            "vals": vals_c,
            "rpeT": rpeT,
            "wq": wq16,
            "bq": bq32,
            "wa": wa16,
            "ba": ba32,
            "stepsf": np.ascontiguousarray(stepsf, dtype=np.float32),
        })
    return in_maps


def kernel(query, keys, vals, rpe_mod, Wq, bq, Wa, ba, steps):
    query = np.asarray(query)
    keys = np.asarray(keys)
    vals = np.asarray(vals)
    rpe_mod = np.asarray(rpe_mod)
    Wq = np.asarray(Wq)
    bq = np.asarray(bq)
    Wa = np.asarray(Wa)
    ba = np.asarray(ba)
    steps = np.asarray(steps)

    core_idx, nbk, nbv, rend = _plan(steps)
    nc = _get_nc(nbk, nbv, rend)
    in_maps = make_in_maps(query, keys, vals, rpe_mod, Wq, bq, Wa, ba, steps)
    res = run_bass_kernel_spmd(nc, in_maps, core_ids=list(range(NCORES)))
    out = np.empty((B, V), dtype=np.float32)
    for c in range(NCORES):
        out[core_idx[c]] = res.results[c]["out"].astype(np.float32)
    return out


# revision 33
# speedup vs baseline: 3.0640x; 1.0380x over previous
"""Trainium2 Bass kernel for the DND retrieval problem.

Full (unsharded) inputs in, full output out. Data-parallel over batch B=64
across 8 NeuronCores (8 batch elements per core), one SPMD Bass program.

Design notes:
- Every large operand ships as fp16 (half the HBM bytes of f32; fp16's
  10-bit mantissa keeps absmax rel err ~3e-3, well under the 2e-2 gate).
- keys are pre-transposed on the host to [k, l] layout so no on-chip
  transposes are needed for the scores matmuls.
- All tensors are SBUF-resident, so every DMA is issued eagerly with no
  waits and the DMA engines stream back-to-back.
- Softmax weights are exactly zero for l >= steps[b], so those (l, b)
  slices of keys/vals are never needed. The host sorts batches by steps
  (descending) and deals them round-robin across cores, so per l-chunk the
  live batches are a prefix and all cores share one live-count profile;
  the program is specialized to that profile (cached per profile) and only
  streams/computes the live prefix of each chunk (~35% fewer bytes for
  uniform steps).

Per-core program (Bc=8, H=8, K=V=256, L=2048):
  qTblk[k, 64]   block-diagonal q (col 8b+h) from wq/query (PE prologue)
  scoresT[bh, l] 2*nb fp16 matmuls per 256-l chunk into a [64,2048] PSUM tile
  softmax over l: scoresT = pscore*rpeT + addmask; global max; ACT Exp with
                  accum rowsums; recip folded back into scoresT
  readT[v, bh]   per 128-l chunk: PE transpose of w + 2*nb tiny (N=8)
                  matmuls accumulating in PSUM across all chunks
  out[b, :]      readT.T @ Wa + ba  (un-permuted on the host)
"""

import numpy as np

import concourse.bacc as bacc
import concourse.bass as bass
import concourse.mybir as mybir
import concourse.tile as tile
from concourse.bass_utils import run_bass_kernel_spmd
from concourse.masks import make_identity
F32 = mybir.dt.float32
F16 = mybir.dt.float16

L = 2048
B = 64
K = 256
V = 256
H = 8
NCORES = 8
BC = B // NCORES          # 8 batch elements per core
NKC = 8                   # keys chunks (256 l each)
KCL = L // NKC            # 256
NVC = 16                  # vals chunks (128 l each)
VCL = L // NVC            # 128
NEG = -1.0e30


def _ap(tensor_ap, offset_elems, dims):
    """Build a raw AP on the same tensor with explicit [step, count] dims."""
    return bass.AP(tensor=tensor_ap.tensor, offset=offset_elems, ap=dims)


def _plan(steps):
    """Sort batches by steps desc, deal round-robin to cores; live-count
    profiles per chunk (max across cores, so one SPMD program fits all)."""
    steps = np.asarray(steps)
    perm = np.argsort(-steps, kind="stable")
    core_idx = [perm[np.arange(BC) * NCORES + c] for c in range(NCORES)]
    nbk = [0] * NKC
    nbv = [0] * NVC
    for c in range(NCORES):
        sc = steps[core_idx[c]]
        for ch in range(NKC):
            nbk[ch] = max(nbk[ch], int((sc > ch * KCL).sum()))
        for vc in range(NVC):
            nbv[vc] = max(nbv[vc], int((sc > vc * VCL).sum()))
    # per batch slot: number of live l-partitions in its LAST live chunk
    # (the rest of that chunk has zero weight and is never loaded/computed)
    rend = [VCL] * BC
    for b in range(BC):
        lvc = max(vc for vc in range(NVC) if nbv[vc] > b)
        r = 1
        for c in range(NCORES):
            s = int(steps[core_idx[c][b]])
            r = max(r, min(s - lvc * VCL, VCL))
        rend[b] = r
    return core_idx, tuple(nbk), tuple(nbv), tuple(rend)


def build_nc(nbk, nbv, rend):
    nc = bacc.Bacc("TRN2", target_bir_lowering=False)

    t_query = nc.dram_tensor("query", [BC, K], F32, kind="ExternalInput").ap()
    t_keysT = nc.dram_tensor("keysT", [NKC, 128, 2, BC, KCL], F16,
                             kind="ExternalInput").ap()
    t_vals = nc.dram_tensor("vals", [NVC, VCL, BC, V], F16,
                            kind="ExternalInput").ap()
    t_rpeT = nc.dram_tensor("rpeT", [B, L], F16, kind="ExternalInput").ap()
    t_wq = nc.dram_tensor("wq", [K, H * K], F16, kind="ExternalInput").ap()
    t_bq = nc.dram_tensor("bq", [H * K], F32, kind="ExternalInput").ap()
    t_wa = nc.dram_tensor("wa", [H * V, V], F16, kind="ExternalInput").ap()
    t_ba = nc.dram_tensor("ba", [V], F32, kind="ExternalInput").ap()
    t_steps = nc.dram_tensor("stepsf", [B], F32, kind="ExternalInput").ap()
    t_out = nc.dram_tensor("out", [BC, V], F32, kind="ExternalOutput").ap()

    with tile.TileContext(nc) as tc:
        _emit(nc, tc, t_query, t_keysT, t_vals, t_rpeT, t_wq, t_bq, t_wa,
              t_ba, t_steps, t_out, nbk, nbv, rend)
    nc.compile()
    return nc


def _emit(nc, tc, t_query, t_keysT, t_vals, t_rpeT, t_wq, t_bq, t_wa, t_ba,
          t_steps, t_out, nbk, nbv, rend):
    from contextlib import ExitStack
    ctx = ExitStack()
    with ctx:
        consts = ctx.enter_context(tc.tile_pool(name="consts", bufs=1))
        keysP = ctx.enter_context(tc.tile_pool(name="keysP", bufs=NKC))
        valsP = ctx.enter_context(tc.tile_pool(name="valsP", bufs=NVC))
        wsbP = ctx.enter_context(tc.tile_pool(name="wsbP", bufs=NVC))
        # PSUM budget is 8 banks: bigP (1) is time-shared by the prologue
        # transposes/q-build and po; pscP (2) double-buffers per-chunk score
        # accumulators so chunk N+1's matmuls don't wait on chunk N's DVE
        # copyback; pwP (3) pipelines the w transposes; prP (2) holds the two
        # readT accumulators (one bank per vh half — interleaved accumulation
        # groups must not share a bank).
        bigP = ctx.enter_context(tc.tile_pool(name="bigP", bufs=1, space="PSUM"))
        pscP = ctx.enter_context(tc.tile_pool(name="pscP", bufs=2, space="PSUM"))
        pwP = ctx.enter_context(tc.tile_pool(name="pwP", bufs=3, space="PSUM"))
        prP = ctx.enter_context(tc.tile_pool(name="prP", bufs=2, space="PSUM"))

        # ------------- DMA issue -------------
        # One deterministic stream on the SP/HWDGE queue (FIFO on the DMA
        # engines): wmat -> rpe -> keys -> ba/wa -> vals. The softmax chain
        # hangs off the LAST keys chunk, so keys go as early as possible;
        # ba/wa hide inside the 15us vals stream; total DMA time is fixed by
        # bytes, only the ordering of the tail matters. Tiny loads ride the
        # Activation HWDGE queue; Pool only builds ident/iota, so the PE
        # prologue is ready before the first keys chunk lands.
        wmat = consts.tile([128, 2, H * K], F16, tag="wmat")
        nc.sync.dma_start(out=wmat, in_=t_wq.rearrange("(a p) j -> p a j", a=2))
        rpeT = consts.tile([64, L], F16, tag="rpeT")
        nc.sync.dma_start(out=rpeT, in_=t_rpeT)

        keys_tiles = []
        for ch in range(NKC):
            nb = nbk[ch]
            if nb == 0:
                keys_tiles.append(None)
                continue
            kt = keysP.tile([128, 2, nb, KCL], F16, tag="keys")
            nc.sync.dma_start(out=kt, in_=t_keysT[ch][:, :, :nb, :])
            keys_tiles.append(kt)

        nbv_next = list(nbv[1:]) + [0]
        vals_tiles = []
        for vc in range(NVC):
            nb = nbv[vc]
            if nb == 0:
                vals_tiles.append(None)
                continue
            nb_full = nbv_next[vc]
            vt = valsP.tile([VCL, nb, V], F16, tag="vals")
            if nb_full > 0:
                nc.sync.dma_start(out=vt[:, :nb_full, :],
                                  in_=t_vals[vc][:, :nb_full, :])
            for b in range(nb_full, nb):
                # partial chunks ride the ACT queue: the SP queue's serial
                # issue rate would otherwise starve the stream tail
                r = rend[b]
                nc.gpsimd.dma_start(out=vt[:r, b:b + 1, :],
                                     in_=t_vals[vc][:r, b:b + 1, :])
            vals_tiles.append(vt)

        # wa/ba stream AFTER vals: they are the latest-needed operands (final
        # projection), so the end-of-stream sem latency lands on them instead
        # of the read-path vals chunks; wa is split so the first half's
        # projection matmuls overlap the second half's transfer
        wa_sb = consts.tile([128, 16, V], F16, tag="wa_sb")
        nc.sync.dma_start(
            out=wa_sb[:, :4, :],
            in_=t_wa.rearrange("(a p) j -> p a j", a=16)[:, :4, :])
        ba_rep = consts.tile([BC, V], F32, tag="ba_rep")
        nc.sync.dma_start(out=ba_rep, in_=_ap(t_ba, 0, [[0, BC], [1, V]]))
        for q in range(1, 4):
            nc.sync.dma_start(
                out=wa_sb[:, 4 * q:4 * q + 4, :],
                in_=t_wa.rearrange("(a p) j -> p a j", a=16)[:, 4 * q:4 * q + 4, :])

        query_sb = consts.tile([BC, K], F32, tag="query")
        nc.scalar.dma_start(out=query_sb, in_=t_query)
        bq_nat = consts.tile([16, 128], F32, tag="bq_nat")
        nc.scalar.dma_start(out=bq_nat, in_=t_bq.rearrange("(r q) -> r q", r=16))
        stepsf = consts.tile([64, 1], F32, tag="stepsf")
        nc.scalar.dma_start(out=stepsf, in_=_ap(t_steps, 0, [[1, 64], [0, 1]]))
        ident = consts.tile([128, 128], F32, tag="ident")
        make_identity(nc, ident)
        iota = consts.tile([64, L], F32, tag="iota")
        nc.gpsimd.iota(iota, pattern=[[1, L]], base=0, channel_multiplier=0,
                       allow_small_or_imprecise_dtypes=True)
        # ------------- prologue compute -------------
        # scoresT starts at NEG: chunks/rows beyond the live prefix are never
        # written by the mults below and must read as fully-masked scores
        scoresT = consts.tile([64, L], F32, tag="scoresT")
        nc.vector.memset(scoresT, NEG)

        # queryT [k, b] (fp16) via PE transpose of query [b, k]
        queryT = consts.tile([128, 2, BC], F16, tag="queryT")
        for half in range(2):
            pq = bigP.tile([128, 256], F32, tag="big")
            nc.tensor.transpose(
                pq[:, :BC], query_sb[:, half * 128:(half + 1) * 128],
                ident[:BC, :BC])
            nc.any.tensor_copy(queryT[:, half, :], pq[:, :BC])

        # bqT [kout, (h,kc)] via PE transpose
        bq_sb = consts.tile([128, 16], F32, tag="bq_sb")
        pb = bigP.tile([128, 256], F32, tag="big")
        nc.tensor.transpose(pb[:, :16], bq_nat, ident[:16, :16])
        nc.any.tensor_copy(bq_sb, pb[:, :16])

        # block-diagonal qT: [kout(128), kc, b, 64 cols]; col 8b+h holds
        # q[b,h,kout], other columns zero, so one matmul per (kc, b)
        # accumulates all 64 (b,h) score rows without cross-terms
        qTblks = []
        for kc in range(2):
            qTblk = consts.tile([128, BC, 64], F16, tag=f"qTblk{kc}",
                                name=f"qTblk{kc}")
            nc.vector.memset(qTblk, 0.0)
            qTblks.append(qTblk)
        # all 16 q matmuls into one PSUM tile first, then all scatters: no
        # per-(kc,h) PE<->ACT ping-pong on a shared buffer
        pq2 = bigP.tile([128, 16, BC], F32, tag="big", name="pq2")
        for kc in range(2):
            for h in range(H):
                idx = kc * H + h
                for kin in range(2):
                    col0 = h * K + kc * 128
                    nc.tensor.matmul(
                        pq2[:, idx, :],
                        lhsT=wmat[:, kin, col0:col0 + 128],
                        rhs=queryT[:, kin, :],
                        start=(kin == 0), stop=(kin == 1),
                        skip_group_check=True,
                    )
        for kc in range(2):
            for h in range(H):
                idx = kc * H + h
                # scatter b -> column 8b+h of batch-b's block (stride 72)
                out_ap = _ap(qTblks[kc], h,
                             [[qTblks[kc].ap[0][0], 128], [72, BC]])
                if kc == 0:
                    nc.scalar.activation(
                        out_ap, pq2[:, idx, :],
                        mybir.ActivationFunctionType.Identity,
                        bias=bq_sb[:, h * 2 + kc:h * 2 + kc + 1], scale=1.0)
                else:
                    nc.vector.tensor_scalar(
                        out=out_ap, in0=pq2[:, idx, :],
                        scalar1=bq_sb[:, h * 2 + kc:h * 2 + kc + 1],
                        scalar2=None, op0=mybir.AluOpType.add)

        # additive -1e30 mask from runtime steps
        addmask = consts.tile([64, L], F32, tag="addmask")
        nc.vector.tensor_scalar(
            out=addmask, in0=iota, scalar1=stepsf, scalar2=NEG,
            op0=mybir.AluOpType.is_ge, op1=mybir.AluOpType.mult)

        # ------------- scores: one [64, 2048] PSUM tile -------------
        # Per chunk: 2*nb matmuls accumulate; rpe modulation is applied on
        # copyback of the live rows; the mask add is fused with a running
        # per-chunk max so everything trails the keys stream and negmax is
        # ready right after the last chunk.
        live_k = [ch for ch in range(NKC) if nbk[ch] > 0]
        runmax = consts.tile([64, NKC], F32, tag="runmax")
        for ch in live_k:
            kt = keys_tiles[ch]
            nb = nbk[ch]
            n_mm = 2 * nb
            i_mm = 0
            pscore = pscP.tile([64, KCL], F32, tag="psc", name="pscore")
            for kc in range(2):
                for b in range(nb):
                    nc.tensor.matmul(
                        pscore,
                        lhsT=qTblks[kc][:, b, :],
                        rhs=kt[:, kc, b, :],
                        start=(i_mm == 0), stop=(i_mm == n_mm - 1))
                    i_mm += 1
            lo = ch * KCL
            nc.vector.tensor_mul(scoresT[:8 * nb, lo:lo + KCL],
                                 pscore[:8 * nb, :],
                                 rpeT[:8 * nb, lo:lo + KCL])
            nc.vector.tensor_add(scoresT[:, lo:lo + KCL],
                                 scoresT[:, lo:lo + KCL],
                                 addmask[:, lo:lo + KCL])
            nc.vector.reduce_max(runmax[:, ch:ch + 1],
                                 scoresT[:, lo:lo + KCL],
                                 axis=mybir.AxisListType.X)

        # ------------- softmax over l (free dim) -------------
        # nbk is non-increasing (batches sorted by steps), so live chunks are
        # a prefix and runmax[:, :n_live] is exactly the written region
        negmax = consts.tile([64, 1], F32, tag="negmax")
        nc.vector.reduce_max(negmax, runmax[:, :len(live_k)],
                             axis=mybir.AxisListType.X, negate=True)
        psums = consts.tile([64, 4], F32, tag="psums")
        for c in range(4):
            lo = c * 512
            nc.scalar.activation(scoresT[:, lo:lo + 512],
                                 scoresT[:, lo:lo + 512],
                                 mybir.ActivationFunctionType.Exp,
                                 bias=negmax, scale=1.0,
                                 accum_out=psums[:, c:c + 1])
        sumexp = consts.tile([64, 1], F32, tag="sumexp")
        nc.vector.reduce_sum(sumexp, psums, axis=mybir.AxisListType.X)
        recip = consts.tile([64, 1], F32, tag="recip")
        nc.vector.reciprocal(recip, sumexp)
        # diag(recip): one regular matmul against it transposes a w chunk AND
        # applies the softmax denominator in the same PE pass (out[l, bh] =
        # sum_r scoresT[r, l] * diag[r, bh] = scoresT[bh, l] * recip[bh])
        dmat = consts.tile([64, 64], F32, tag="dmat")
        nc.vector.tensor_scalar(
            out=dmat, in0=ident[:64, :64], scalar1=recip, scalar2=None,
            op0=mybir.AluOpType.mult)

        # ------------- read: accumulate readT[v, bh] over all l -------------
        # lastvc[b]: the last chunk where batch-slot b is live (per-column
        # accumulation groups need their stop on their own final matmul).
        # The transpose+normalize matmuls (PE) are emitted one chunk ahead of
        # the read matmuls so PE never stalls on the w_sb copyback.
        lastvc = [max(vc for vc in range(NVC) if nbv[vc] > b)
                  for b in range(BC)]
        live_v = [vc for vc in range(NVC) if nbv[vc] > 0]
        preadT = [prP.tile([128, 64], F32, tag="pr", name=f"preadT{vh}")
                  for vh in range(2)]

        pw_tiles = {}

        def emit_wT(vc):
            pw = pwP.tile([128, 64], F32, tag="pw")
            off = vc * VCL
            nc.tensor.matmul(pw, lhsT=scoresT[:, off:off + VCL],
                             rhs=dmat, start=True, stop=True)
            pw_tiles[vc] = pw

        emit_wT(live_v[0])
        for i, vc in enumerate(live_v):
            if i + 1 < len(live_v):
                emit_wT(live_v[i + 1])
            vt = vals_tiles[vc]
            nb = nbv[vc]
            w_sb = wsbP.tile([128, 64], F16, tag="wsb")
            cb = nc.vector.tensor_copy if i % 2 == 0 else nc.scalar.copy
            cb(w_sb, pw_tiles.pop(vc))
            for vh in range(2):
                for b in range(nb):
                    r = rend[b] if vc == lastvc[b] else VCL
                    nc.tensor.matmul(
                        preadT[vh][:, 8 * b:8 * b + 8],
                        lhsT=vt[:r, b, vh * 128:(vh + 1) * 128],
                        rhs=w_sb[:r, 8 * b:8 * b + 8],
                        start=(vc == live_v[0] and b == 0),
                        stop=(vc == lastvc[b]),
                        skip_group_check=True)

        # ------------- epilogue: head aggregation + store -------------
        readT_sb = consts.tile([128, 2, 64], F16, tag="readT_sb")
        nc.vector.tensor_copy(readT_sb[:, 0, :], preadT[0])
        nc.scalar.copy(readT_sb[:, 1, :], preadT[1])

        po = bigP.tile([64, V], F32, tag="big", name="po")
        n_mm = 2 * H
        i_mm = 0
        for h in range(H):      # wa-chunk order h*2+half: 0..15
            for half in range(2):
                lhsT = _ap(readT_sb, half * 64 + h,
                           [[readT_sb.ap[0][0], 128], [8, BC]])
                nc.tensor.matmul(
                    po[:BC, :], lhsT=lhsT, rhs=wa_sb[:, h * 2 + half, :],
                    start=(i_mm == 0), stop=(i_mm == n_mm - 1))
                i_mm += 1
        out_sb = consts.tile([BC, V], F32, tag="out_sb")
        nc.vector.tensor_add(out_sb, po[:BC, :], ba_rep)
        nc.sync.dma_start(out=t_out, in_=out_sb)


_NC_CACHE = {}
_LAST_NC = None


def _get_nc(nbk=None, nbv=None, rend=None):
    global _LAST_NC
    if nbk is None:
        # test/profiling convenience: the program from the latest kernel()
        # call (or the untruncated profile if none was made yet)
        if _LAST_NC is None:
            return _get_nc((BC,) * NKC, (BC,) * NVC, (VCL,) * BC)
        return _LAST_NC
    key = (nbk, nbv, rend)
    if key not in _NC_CACHE:
        _NC_CACHE[key] = build_nc(nbk, nbv, rend)
    _LAST_NC = _NC_CACHE[key]
    return _LAST_NC


def make_in_maps(query, keys, vals, rpe_mod, Wq, bq, Wa, ba, steps):
    core_idx = _plan(steps)[0]
    wq16 = np.ascontiguousarray(Wq, dtype=np.float16)
    wa16 = np.ascontiguousarray(Wa, dtype=np.float16)
    bq32 = np.ascontiguousarray(bq, dtype=np.float32)
    ba32 = np.ascontiguousarray(ba, dtype=np.float32)
    rpe = np.asarray(rpe_mod)[:, :, 0]  # [L, B]
    in_maps = []
    for c in range(NCORES):
        bs = core_idx[c]
        # keysT[ch, kp, kc, b, l] = keys[ch*256 + l, b, kc*128 + kp]
        kc_ = np.asarray(keys[:, bs, :]).reshape(NKC, KCL, BC, 2, 128)
        keysT = np.ascontiguousarray(
            kc_.transpose(0, 4, 3, 2, 1), dtype=np.float16)
        vals_c = np.ascontiguousarray(
            np.asarray(vals[:, bs, :]).reshape(NVC, VCL, BC, V),
            dtype=np.float16)
        rpeT = np.ascontiguousarray(
            np.repeat(rpe[:, bs].T, H, axis=0), dtype=np.float16)
        stepsf = np.repeat(
            np.asarray(steps[bs]).astype(np.float32), H)
        in_maps.append({
            "query": np.ascontiguousarray(query[bs], dtype=np.float32),
            "keysT": keysT,
            "vals": vals_c,
            "rpeT": rpeT,
            "wq": wq16,
            "bq": bq32,
            "wa": wa16,
            "ba": ba32,
            "stepsf": np.ascontiguousarray(stepsf, dtype=np.float32),
        })
    return in_maps


def kernel(query, keys, vals, rpe_mod, Wq, bq, Wa, ba, steps):
    query = np.asarray(query)
    keys = np.asarray(keys)
    vals = np.asarray(vals)
    rpe_mod = np.asarray(rpe_mod)
    Wq = np.asarray(Wq)
    bq = np.asarray(bq)
    Wa = np.asarray(Wa)
    ba = np.asarray(ba)
    steps = np.asarray(steps)

    core_idx, nbk, nbv, rend = _plan(steps)
    nc = _get_nc(nbk, nbv, rend)
    in_maps = make_in_maps(query, keys, vals, rpe_mod, Wq, bq, Wa, ba, steps)
    res = run_bass_kernel_spmd(nc, in_maps, core_ids=list(range(NCORES)))
    out = np.empty((B, V), dtype=np.float32)
    for c in range(NCORES):
        out[core_idx[c]] = res.results[c]["out"].astype(np.float32)
    return out
